# revision 11
# baseline (speedup 1.0000x reference)
"""CardHistorySAGE Trainium2 kernel (8-core SPMD, no collectives).

Strategy
--------
Host: sort hist rows by card id, walk cards into fixed-shape "windows"
(<= RPW rows, <= 128 cards each, rows padded to exactly RPW).  Consecutive
windows are dealt to the 8 cores in equal blocks, so every card's history
rows AND every target referencing that card land on the same core: the
per-card segment-mean, card MLP, gather and head are all core-local.

All data-dependent decisions (row permutation, in-window card slots,
1/count scales, target->chunk packing) are baked into *input tensors*;
the instruction stream is identical on all 8 cores (single SPMD program).

Device (per core):
  hist MLP  : feature-major 2-layer MLP, N=512 matmul streams
  segment   : per 128-row chunk, one-hot SegMat (DVE is_equal) @ rows
              accumulated in PSUM per 128-card window
  card MLP  : scale by host 1/count, transpose, 2-layer MLP over card slots
  gather    : selection-matrix matmuls pull each target's card vector
  head      : 2-layer head -> logits
Host: scatter per-core logits back to the original target order.
"""

import os
import numpy as np

from concourse import bass, bacc, mybir
import concourse.tile as tile

F32 = mybir.dt.float32

P = 128          # partitions / chunk rows
D = 128          # hidden dim
CPW = 7          # chunks per window
RPW = CPW * P    # rows per window (896)
TGT_J = 4        # card-windows covered by one target chunk (512 slots)
TSTRIDE = 256    # slot stride between target chunks (2 windows)
NCORES = 8


# ----------------------------------------------------------------- planning

def _plan(target_x, hist_x, hist_idx, target_idx, card_dense,
          W_c1, b_c1):
    """Builds per-core input tensors + schedule metadata."""
    B, F = target_x.shape
    H = hist_x.shape[0]
    U, CF = card_dense.shape

    counts = np.bincount(hist_idx, minlength=U).astype(np.int64)
    assert counts.max() <= RPW, "single card exceeds one window"
    order = np.argsort(hist_idx, kind="stable")
    sidx = hist_idx[order]
    cum = np.concatenate([[0], np.cumsum(counts)])  # rows before card c

    # walk cards into windows
    w_c0 = []
    w_c1_ = []
    c0 = 0
    while c0 < U:
        hi = np.searchsorted(cum, cum[c0] + RPW, side="right") - 1
        c1 = min(hi, c0 + P, U)
        assert c1 > c0
        w_c0.append(c0)
        w_c1_.append(c1)
        c0 = c1
    nwin = len(w_c0)
    w_c0 = np.array(w_c0)
    w_c1_ = np.array(w_c1_)

    Wpc = -(-nwin // NCORES)          # real windows dealt per core
    Wc = -(-(Wpc + 1) // 8) * 8       # +1: local window 0 is a reserved pad
    assert Wc - 1 >= Wpc
    C = Wc * CPW                      # chunks per core
    T = C * P // 512                  # hist 512-row tiles per core
    S = Wc * P                        # card slots per core
    NTCH = Wc // 2                    # target chunks per core
    BKP = NTCH * P                    # padded targets per core
    TT = BKP // 512                   # target 512 tiles / head groups

    win_of_card = np.zeros(U, np.int64)
    slot_of_card = np.zeros(U, np.int64)
    for w in range(nwin):
        win_of_card[w_c0[w]:w_c1_[w]] = w
        slot_of_card[w_c0[w]:w_c1_[w]] = np.arange(w_c1_[w] - w_c0[w])

    in_maps = []
    scatter = []   # (orig_ids, positions) per core
    for k in range(NCORES):
        hx_pad = np.zeros((Wc * RPW, F), np.float32)
        slot_pad = np.full(Wc * RPW, -1.0, np.float32)
        cnt_slot = np.zeros(S, np.float32)
        cdf_slot = np.zeros((S, CF), np.float32)
        for wl in range(1, Wc):
            w = k * Wpc + (wl - 1)
            if w >= min((k + 1) * Wpc, nwin):
                continue
            a, b = w_c0[w], w_c1_[w]
            r0, r1 = cum[a], cum[b]
            n = r1 - r0
            hx_pad[wl * RPW: wl * RPW + n] = hist_x[order[r0:r1]]
            slot_pad[wl * RPW: wl * RPW + n] = (sidx[r0:r1] - a).astype(np.float32)
            width = b - a
            cnt_slot[wl * P: wl * P + width] = counts[a:b]
            cdf_slot[wl * P: wl * P + width] = card_dense[a:b]
        hx_tiles = np.ascontiguousarray(
            hx_pad.reshape(T, 512, F).transpose(0, 2, 1))
        slot2d = np.ascontiguousarray(slot_pad.reshape(C, P).T)
        inv_sb = np.ascontiguousarray(
            (1.0 / np.maximum(cnt_slot, 1.0)).reshape(Wc, P).T)
        pre = cdf_slot @ W_c1[:CF] + b_c1          # [S, D] host K=CF matmul
        pre_tiles = np.ascontiguousarray(
            pre.T.reshape(D, S // 512, 512).transpose(1, 0, 2)).astype(np.float32)

        # ---- targets of this core
        tw = win_of_card[target_idx]
        ids = np.nonzero((tw >= k * Wpc) & (tw < min((k + 1) * Wpc, nwin)))[0]
        slots_abs = (tw[ids] - k * Wpc + 1) * P + slot_of_card[target_idx[ids]]
        t_ord = np.argsort(slots_abs, kind="stable")
        ids = ids[t_ord]
        slots_abs = slots_abs[t_ord]
        # EDF greedy: earliest feasible chunk (chunk c covers [256c, 256c+512))
        used = np.zeros(NTCH, np.int32)
        chunk_of = np.empty(len(ids), np.int64)
        for i, s in enumerate(slots_abs):
            lo = max(0, (s - TGT_J * P + TSTRIDE) // TSTRIDE)  # smallest c with 256c+512 > s
            hi = min(s // TSTRIDE, NTCH - 1)
            c = lo
            while c <= hi and used[c] >= P:
                c += 1
            assert c <= hi, "target chunk packing failed"
            chunk_of[i] = c
            used[c] += 1
        pos_in = np.zeros(len(ids), np.int64)
        fill = np.zeros(NTCH, np.int64)
        for i in range(len(ids)):
            c = chunk_of[i]
            pos_in[i] = c * P + fill[c]
            fill[c] += 1
        tx_pad = np.zeros((BKP, F), np.float32)
        trel = np.full((1, BKP), -999.0, np.float32)
        tx_pad[pos_in] = target_x[ids]
        trel[0, pos_in] = (slots_abs - TSTRIDE * chunk_of).astype(np.float32)
        tx_tiles = np.ascontiguousarray(
            tx_pad.reshape(TT, 512, F).transpose(0, 2, 1))

        in_maps.append(dict(
            hx=hx_tiles, slot=slot2d, inv=inv_sb, pre=pre_tiles,
            tx=tx_tiles, trel=trel,
        ))
        scatter.append((ids, pos_in))

    meta = dict(Wc=Wc, C=C, T=T, S=S, NTCH=NTCH, BKP=BKP, TT=TT, F=F, CF=CF)
    return in_maps, scatter, meta


# ------------------------------------------------------------- the program

def _build(meta, weights):
    Wc, C, T, S = meta["Wc"], meta["C"], meta["T"], meta["S"]
    NTCH, BKP, TT, F = meta["NTCH"], meta["BKP"], meta["TT"], meta["F"]
    NCG = Wc // 4            # card-mlp groups (4 windows each)

    nc = bacc.Bacc(None)
    dp = nc.declare_dram_parameter
    hx_d = dp("hx", [T, F, 512], F32, isOutput=False)
    slot_d = dp("slot", [P, C], F32, isOutput=False)
    inv_d = dp("inv", [P, Wc], F32, isOutput=False)
    pre_d = dp("pre", [S // 512, D, 512], F32, isOutput=False)
    tx_d = dp("tx", [TT, F, 512], F32, isOutput=False)
    trel_d = dp("trel", [1, BKP], F32, isOutput=False)
    wd = {}
    for name, arr in weights.items():
        wd[name] = dp(name, list(arr.shape), F32, isOutput=False)
    out_d = dp("logits", [1, BKP], F32, isOutput=True)

    with tile.TileContext(nc) as tc:
        import contextlib
        with contextlib.ExitStack() as ctx:
            const = ctx.enter_context(tc.tile_pool(name="const", bufs=1))
            big = ctx.enter_context(tc.tile_pool(name="big", bufs=1))
            hxp = ctx.enter_context(tc.tile_pool(name="hxp", bufs=3))
            sb1 = ctx.enter_context(tc.tile_pool(name="sb1", bufs=2))
            sb2 = ctx.enter_context(tc.tile_pool(name="sb2", bufs=2))
            hrmp = ctx.enter_context(tc.tile_pool(name="hrmp", bufs=4))
            segp = ctx.enter_context(tc.tile_pool(name="segp", bufs=4))
            aggp = ctx.enter_context(tc.tile_pool(name="aggp", bufs=2))
            prep = ctx.enter_context(tc.tile_pool(name="prep", bufs=2))
            cwp = ctx.enter_context(tc.tile_pool(name="cwp", bufs=2))
            pbig = ctx.enter_context(
                tc.tile_pool(name="pbig", bufs=2, space="PSUM"))
            ptr = ctx.enter_context(
                tc.tile_pool(name="ptr", bufs=2, space="PSUM"))

            # ---- constants to SBUF
            def load_const(name, shape):
                t = const.tile(list(shape), F32, tag=name)
                nc.sync.dma_start(out=t[:], in_=wd[name][:])
                return t
            wt1 = load_const("W_t1", (F, D))
            bt1 = load_const("b_t1", (P, 1))
            wt2 = load_const("W_t2", (D, D))
            bt2 = load_const("b_t2", (P, 1))
            wc1b = load_const("W_c1b", (D, D))
            wc2 = load_const("W_c2", (D, D))
            bc2 = load_const("b_c2", (P, 1))
            wh1a = load_const("W_h1a", (D, D))
            wh1b = load_const("W_h1b", (D, D))
            bh1 = load_const("b_h1", (P, 1))
            wh2 = load_const("W_h2", (D, 1))
            ident = load_const("ident", (P, P))
            iota_r = load_const("iota_r", (P, P))
            iota_c = load_const("iota_c", (P, 1))
            ones_r = load_const("ones_r", (1, P))

            slot_sb = const.tile([P, C], F32, tag="slot")
            nc.sync.dma_start(out=slot_sb[:], in_=slot_d[:])
            inv_sb = const.tile([P, Wc], F32, tag="inv")
            nc.sync.dma_start(out=inv_sb[:], in_=inv_d[:])


            card_h = big.tile([P, (Wc + 2) * P], F32, tag="card_h")
            tgt_h = big.tile([P, BKP], F32, tag="tgt_h")
            nc.vector.memset(card_h[:, Wc * P:], 0.0)

            relu = mybir.ActivationFunctionType.Relu
            fcopy = mybir.ActivationFunctionType.Copy

            # ---------------- card-MLP group (windows 4g..4g+3)
            def card_group(g, agg_tile):
                aggT = cwp.tile([P, 512], F32, tag="aggT")
                for wr in range(4):
                    ptt = ptr.tile([P, P], F32, tag="pt")
                    nc.tensor.transpose(
                        out=ptt[:], in_=agg_tile[:, wr * P:(wr + 1) * P],
                        identity=ident[:])
                    nc.vector.tensor_copy(
                        out=aggT[:, wr * P:(wr + 1) * P], in_=ptt[:])
                pre_sb = prep.tile([D, 512], F32, tag="pre")
                nc.sync.dma_start(out=pre_sb[:], in_=pre_d[g])
                pc1 = pbig.tile([P, 512], F32, tag="pb")
                nc.tensor.matmul(out=pc1[:], lhsT=wc1b[:], rhs=aggT[:],
                                 start=True, stop=True)
                c1 = cwp.tile([P, 512], F32, tag="c1")
                nc.vector.tensor_tensor(out=c1[:], in0=pc1[:], in1=pre_sb[:],
                                        op=mybir.AluOpType.add)
                nc.scalar.activation(c1[:], c1[:], relu)
                pc2 = pbig.tile([P, 512], F32, tag="pb")
                nc.tensor.matmul(out=pc2[:], lhsT=wc2[:], rhs=c1[:],
                                 start=True, stop=True)
                chT = cwp.tile([P, 512], F32, tag="chT")
                nc.scalar.activation(chT[:], pc2[:], relu, bias=bc2[:, 0:1])
                for wr in range(4):
                    w = 4 * g + wr
                    ptt = ptr.tile([P, P], F32, tag="pt")
                    nc.tensor.transpose(
                        out=ptt[:], in_=chT[:, wr * P:(wr + 1) * P],
                        identity=ident[:])
                    nc.vector.tensor_copy(
                        out=card_h[:, w * P:(w + 1) * P], in_=ptt[:])

            # ---------------- hist loop
            with tc.tile_pool(name="pseg", bufs=2, space="PSUM") as pseg:
                win_psum = None
                agg_tile = None
                for t in range(T):
                    hx_sb = hxp.tile([F, 512], F32, tag="hx")
                    nc.sync.dma_start(out=hx_sb[:], in_=hx_d[t])
                    ph1 = pbig.tile([P, 512], F32, tag="pb")
                    nc.tensor.matmul(out=ph1[:], lhsT=wt1[:], rhs=hx_sb[:],
                                     start=True, stop=True)
                    h1 = sb1.tile([P, 512], F32, tag="h1")
                    nc.scalar.activation(h1[:], ph1[:], relu, bias=bt1[:, 0:1])
                    ph2 = pbig.tile([P, 512], F32, tag="pb")
                    nc.tensor.matmul(out=ph2[:], lhsT=wt2[:], rhs=h1[:],
                                     start=True, stop=True)
                    h2 = sb2.tile([P, 512], F32, tag="h2")
                    nc.scalar.activation(h2[:], ph2[:], relu, bias=bt2[:, 0:1])
                    for kk in range(4):
                        chunk = 4 * t + kk
                        w, ci = divmod(chunk, CPW)
                        ptt = ptr.tile([P, P], F32, tag="pt")
                        nc.tensor.transpose(
                            out=ptt[:], in_=h2[:, kk * P:(kk + 1) * P],
                            identity=ident[:])
                        hrm = hrmp.tile([P, P], F32, tag="hrm")
                        nc.vector.tensor_copy(out=hrm[:], in_=ptt[:])
                        seg = segp.tile([P, P], F32, tag="seg")
                        nc.vector.tensor_tensor(
                            out=seg[:],
                            in0=slot_sb[:, chunk:chunk + 1].to_broadcast([P, P]),
                            in1=iota_r[:],
                            op=mybir.AluOpType.is_equal)
                        if ci == 0:
                            win_psum = pseg.tile([P, P], F32, tag="win")
                        nc.tensor.matmul(out=win_psum[:], lhsT=seg[:],
                                         rhs=hrm[:],
                                         start=(ci == 0), stop=(ci == CPW - 1))
                        if ci == CPW - 1:
                            wr = w % 4
                            if wr == 0:
                                agg_tile = aggp.tile([P, 512], F32, tag="agg")
                            nc.scalar.activation(
                                agg_tile[:, wr * P:(wr + 1) * P], win_psum[:],
                                fcopy, scale=inv_sb[:, w:w + 1])
                            if wr == 3:
                                card_group(w // 4, agg_tile)

            # ---------------- target MLP
            for t in range(TT):
                tx_sb = hxp.tile([F, 512], F32, tag="hx")
                nc.sync.dma_start(out=tx_sb[:], in_=tx_d[t])
                ph1 = pbig.tile([P, 512], F32, tag="pb")
                nc.tensor.matmul(out=ph1[:], lhsT=wt1[:], rhs=tx_sb[:],
                                 start=True, stop=True)
                h1 = sb1.tile([P, 512], F32, tag="h1")
                nc.scalar.activation(h1[:], ph1[:], relu, bias=bt1[:, 0:1])
                ph2 = pbig.tile([P, 512], F32, tag="pb")
                nc.tensor.matmul(out=ph2[:], lhsT=wt2[:], rhs=h1[:],
                                 start=True, stop=True)
                nc.scalar.activation(tgt_h[:, t * 512:(t + 1) * 512], ph2[:],
                                     relu, bias=bt2[:, 0:1])

            # ---------------- gather + head
            with (tc.tile_pool(name="pgat", bufs=2, space="PSUM") as pgat,
                  tc.tile_pool(name="plog", bufs=2, space="PSUM") as plog,
                  tc.tile_pool(name="headp", bufs=2) as headp,
                  tc.tile_pool(name="selp", bufs=4) as selp):
                for g in range(TT):
                    trg = headp.tile([1, 512], F32, tag="trg")
                    nc.sync.dma_start(
                        out=trg[:], in_=trel_d[0:1, g * 512:(g + 1) * 512])
                    ptb = pbig.tile([P, 512], F32, tag="pb")
                    nc.tensor.matmul(
                        out=ptb[:], lhsT=ones_r[:], rhs=trg[:],
                        start=True, stop=True)
                    tb = headp.tile([P, 512], F32, tag="tb")
                    nc.vector.tensor_copy(out=tb[:], in_=ptb[:])
                    tch = headp.tile([P, 512], F32, tag="tch")
                    for kk in range(4):
                        c = 4 * g + kk
                        pg = pgat.tile([P, P], F32, tag="pg")
                        for j in range(TGT_J):
                            sel = selp.tile([P, P], F32, tag="sel")
                            nc.vector.tensor_scalar(
                                out=sel[:],
                                in0=tb[:, kk * P:(kk + 1) * P],
                                scalar1=iota_c[:, 0:1],
                                scalar2=float(P * j),
                                op0=mybir.AluOpType.subtract,
                                op1=mybir.AluOpType.is_equal)
                            nc.tensor.matmul(
                                out=pg[:],
                                lhsT=card_h[:, (2 * c + j) * P:(2 * c + j + 1) * P],
                                rhs=sel[:],
                                start=(j == 0), stop=(j == TGT_J - 1))
                        nc.vector.tensor_copy(
                            out=tch[:, kk * P:(kk + 1) * P], in_=pg[:])
                    phh = pbig.tile([P, 512], F32, tag="pb")
                    nc.tensor.matmul(out=phh[:], lhsT=wh1a[:],
                                     rhs=tgt_h[:, g * 512:(g + 1) * 512],
                                     start=True, stop=False)
                    nc.tensor.matmul(out=phh[:], lhsT=wh1b[:], rhs=tch[:],
                                     start=False, stop=True)
                    hh = headp.tile([P, 512], F32, tag="hh")
                    nc.scalar.activation(hh[:], phh[:], relu, bias=bh1[:, 0:1])
                    pl = plog.tile([1, 512], F32, tag="pl")
                    nc.tensor.matmul(out=pl[:], lhsT=wh2[:], rhs=hh[:],
                                     start=True, stop=True)
                    ol = headp.tile([1, 512], F32, tag="ol")
                    nc.scalar.activation(ol[:], pl[:], fcopy,
                                         bias=float(weights["b_h2"][0, 0]))
                    nc.sync.dma_start(
                        out=out_d[0:1, g * 512:(g + 1) * 512], in_=ol[:])
    nc.finalize()
    return nc


# ------------------------------------------------------------------ driver

def _weights_dict(W_t1, b_t1, W_t2, b_t2, W_c1, b_c1, W_c2, b_c2,
                  W_h1, b_h1, W_h2, b_h2, CF):
    col = lambda v: np.ascontiguousarray(v.reshape(-1, 1).astype(np.float32))
    iota = np.arange(P, dtype=np.float32)
    return dict(
        W_t1=np.ascontiguousarray(W_t1, np.float32),
        b_t1=col(b_t1),
        W_t2=np.ascontiguousarray(W_t2, np.float32),
        b_t2=col(b_t2),
        W_c1b=np.ascontiguousarray(W_c1[CF:], np.float32),
        W_c2=np.ascontiguousarray(W_c2, np.float32),
        b_c2=col(b_c2),
        W_h1a=np.ascontiguousarray(W_h1[:D], np.float32),
        W_h1b=np.ascontiguousarray(W_h1[D:], np.float32),
        b_h1=col(b_h1),
        W_h2=np.ascontiguousarray(W_h2, np.float32),
        b_h2=np.array([[float(np.asarray(b_h2).reshape(-1)[0])]], np.float32),
        ident=np.eye(P, dtype=np.float32),
        iota_r=np.tile(iota[None, :], (P, 1)),
        iota_c=col(iota),
        ones_r=np.ones((1, P), np.float32),
    )


def _prepare(inputs):
    gi = lambda n: np.asarray(inputs[n])
    target_x = gi("target_x").astype(np.float32)
    hist_x = gi("hist_x").astype(np.float32)
    hist_idx = gi("hist_card_local_idx").astype(np.int64)
    target_idx = gi("target_card_local_idx").astype(np.int64)
    card_dense = gi("card_dense_feats").astype(np.float32)
    CF = card_dense.shape[1]
    weights = _weights_dict(
        gi("W_t1"), gi("b_t1"), gi("W_t2"), gi("b_t2"),
        gi("W_c1"), gi("b_c1"), gi("W_c2"), gi("b_c2"),
        gi("W_h1"), gi("b_h1"), gi("W_h2"), gi("b_h2"), CF)
    in_maps, scatter, meta = _plan(
        target_x, hist_x, hist_idx, target_idx, card_dense,
        gi("W_c1").astype(np.float32), gi("b_c1").astype(np.float32))
    for m in in_maps:
        m.update(weights)
    return in_maps, scatter, meta, weights, target_x.shape[0]


def kernel(**inputs):
    in_maps, scatter, meta, weights, B = _prepare(inputs)
    nc = _build(meta, weights)

    if os.environ.get("BASS_KERNEL_SIM"):
        from concourse import bass_interp
        outs = []
        for k in range(NCORES):
            sim = bass_interp.CoreSim(nc)
            for name, arr in in_maps[k].items():
                sim.tensor(name)[:] = arr
            sim.simulate()
            outs.append(np.array(sim.tensor("logits")))
    else:
        from concourse.bass_utils import run_bass_kernel_spmd
        res = run_bass_kernel_spmd(nc, in_maps, list(range(NCORES)))
        outs = [res.results[k]["logits"] for k in range(NCORES)]

    out = np.zeros(B, np.float32)
    for k in range(NCORES):
        ids, pos = scatter[k]
        out[ids] = outs[k].reshape(-1)[pos]
    return out


# revision 12
# speedup vs baseline: 1.1156x; 1.1156x over previous
"""CardHistorySAGE Trainium2 kernel (8-core SPMD, no collectives).

Strategy
--------
Host: sort hist rows by card id, walk cards into fixed-shape "windows"
(<= RPW rows, <= 128 cards each, rows padded to exactly RPW).  Consecutive
windows are dealt to the 8 cores in equal blocks, so every card's history
rows AND every target referencing that card land on the same core: the
per-card segment-mean, card MLP, gather and head are all core-local.

All data-dependent decisions (row permutation, in-window card slots,
1/count row scales, target->chunk packing) are baked into *input
tensors*; the instruction stream is identical on all 8 cores.

Device (per core), all fp32:
  hist MLP   : mm1 feature-major (K=64 row-half packing), mm2 row-major
               (lhsT = h1 chunks) -> row-major h in PSUM, fused
               relu*inv_count on DVE
  segment    : one-hot SegMat (batched DVE is_equal); seg matmul
               lhsT=h_rows rhs=SegMat accumulates feature-major per-card
               sums for 4 windows per PSUM bank
  card MLP   : mm1 feature-major + host-precomputed dense-feat term,
               mm2 row-major -> row-major card_h table in SBUF
  gather     : selection-matrix matmuls pull targets' card vectors
  head       : feature-major 2-layer head -> logits
Host: scatter per-core logits back to the original target order.
"""

import os
import numpy as np

from concourse import bass, bacc, mybir
import concourse.tile as tile

F32 = mybir.dt.float32

P = 128          # partitions / chunk rows
D = 128          # hidden dim
CPW = 7          # chunks per window
RPW = CPW * P    # rows per window (896)
TGT_J = 4        # card-windows covered by one target chunk (512 slots)
TSTRIDE = 256    # slot stride between target chunks (2 windows)
NCORES = 8
MACT = 4         # hist/target 512-row tiles per DMA macro


def _pack_tiles(tiles):
    """[N, 64, 512] f32 -> [ceil(N/4), 128, 1024]; tile t at partition-half
    (t%4)%2, column-chunk (t%4)//2 of macro t//4."""
    N = tiles.shape[0]
    NM = -(-N // MACT)
    out = np.zeros((NM, 128, 1024), np.float32)
    for t in range(N):
        j = t % MACT
        out[t // MACT, (j % 2) * 64:(j % 2) * 64 + 64,
            (j // 2) * 512:(j // 2) * 512 + 512] = tiles[t]
    return out


# ----------------------------------------------------------------- planning

def _plan(target_x, hist_x, hist_idx, target_idx, card_dense,
          W_c1, b_c1):
    """Builds per-core input tensors + schedule metadata."""
    B, F = target_x.shape
    U, CF = card_dense.shape

    counts = np.bincount(hist_idx, minlength=U).astype(np.int64)
    assert counts.max() <= RPW, "single card exceeds one window"
    order = np.argsort(hist_idx, kind="stable")
    sidx = hist_idx[order]
    cum = np.concatenate([[0], np.cumsum(counts)])  # rows before card c

    # walk cards into windows
    w_c0 = []
    w_c1_ = []
    c0 = 0
    while c0 < U:
        hi = np.searchsorted(cum, cum[c0] + RPW, side="right") - 1
        c1 = min(hi, c0 + P, U)
        assert c1 > c0
        w_c0.append(c0)
        w_c1_.append(c1)
        c0 = c1
    nwin = len(w_c0)
    w_c0 = np.array(w_c0)
    w_c1_ = np.array(w_c1_)

    Wpc = -(-nwin // NCORES)          # real windows dealt per core
    Wc = -(-(Wpc + 1) // 8) * 8       # +1: local window 0 is a reserved pad
    assert Wc - 1 >= Wpc
    C = Wc * CPW                      # chunks per core
    T = C * P // 512                  # hist 512-row tiles per core
    S = Wc * P                        # card slots per core
    NTCH = Wc // 2                    # target chunks per core
    BKP = NTCH * P                    # padded targets per core
    TT = BKP // 512                   # target 512 tiles / head groups

    win_of_card = np.zeros(U, np.int64)
    slot_of_card = np.zeros(U, np.int64)
    for w in range(nwin):
        win_of_card[w_c0[w]:w_c1_[w]] = w
        slot_of_card[w_c0[w]:w_c1_[w]] = np.arange(w_c1_[w] - w_c0[w])

    in_maps = []
    scatter = []   # (orig_ids, positions) per core
    for k in range(NCORES):
        hx_pad = np.zeros((Wc * RPW, F), np.float32)
        slot_pad = np.full(Wc * RPW, -1.0, np.float32)
        rinv_pad = np.zeros(Wc * RPW, np.float32)
        cdf_slot = np.zeros((S, CF), np.float32)
        for wl in range(1, Wc):
            w = k * Wpc + (wl - 1)
            if w >= min((k + 1) * Wpc, nwin):
                continue
            a, b = w_c0[w], w_c1_[w]
            r0, r1 = cum[a], cum[b]
            n = r1 - r0
            hx_pad[wl * RPW: wl * RPW + n] = hist_x[order[r0:r1]]
            slot_pad[wl * RPW: wl * RPW + n] = (sidx[r0:r1] - a).astype(np.float32)
            rinv_pad[wl * RPW: wl * RPW + n] = (
                1.0 / counts[sidx[r0:r1]]).astype(np.float32)
            cdf_slot[wl * P: wl * P + (b - a)] = card_dense[a:b]
        hx_tiles = _pack_tiles(hx_pad.reshape(T, 512, F).transpose(0, 2, 1))
        slot2d = np.ascontiguousarray(slot_pad.reshape(C, P).T)
        rinv2d = np.ascontiguousarray(rinv_pad.reshape(C, P).T)
        pre = (cdf_slot @ W_c1[:CF] + b_c1).astype(np.float32)  # [S, D]
        pre_tiles = np.ascontiguousarray(
            pre.T.reshape(D, S // 1024, 1024).transpose(1, 0, 2))

        # ---- targets of this core
        tw = win_of_card[target_idx]
        ids = np.nonzero((tw >= k * Wpc) & (tw < min((k + 1) * Wpc, nwin)))[0]
        slots_abs = (tw[ids] - k * Wpc + 1) * P + slot_of_card[target_idx[ids]]
        t_ord = np.argsort(slots_abs, kind="stable")
        ids = ids[t_ord]
        slots_abs = slots_abs[t_ord]
        # EDF greedy: earliest feasible chunk (chunk c covers [256c, 256c+512))
        used = np.zeros(NTCH, np.int32)
        chunk_of = np.empty(len(ids), np.int64)
        for i, s in enumerate(slots_abs):
            lo = max(0, (s - TGT_J * P + TSTRIDE) // TSTRIDE)
            hi = min(s // TSTRIDE, NTCH - 1)
            c = lo
            while c <= hi and used[c] >= P:
                c += 1
            assert c <= hi, "target chunk packing failed"
            chunk_of[i] = c
            used[c] += 1
        pos_in = np.zeros(len(ids), np.int64)
        fill = np.zeros(NTCH, np.int64)
        for i in range(len(ids)):
            c = chunk_of[i]
            pos_in[i] = c * P + fill[c]
            fill[c] += 1
        tx_pad = np.zeros((BKP, F), np.float32)
        trel = np.full((1, BKP), -999.0, np.float32)
        tx_pad[pos_in] = target_x[ids]
        trel[0, pos_in] = (slots_abs - TSTRIDE * chunk_of).astype(np.float32)
        tx_tiles = _pack_tiles(tx_pad.reshape(TT, 512, F).transpose(0, 2, 1))

        in_maps.append(dict(
            hx=hx_tiles, slot=slot2d, rinv=rinv2d, pre=pre_tiles,
            tx=tx_tiles, trel=trel,
        ))
        scatter.append((ids, pos_in))

    meta = dict(Wc=Wc, C=C, T=T, S=S, NTCH=NTCH, BKP=BKP, TT=TT, F=F, CF=CF,
                NMH=hx_tiles.shape[0], NMT=tx_tiles.shape[0])
    return in_maps, scatter, meta


# ------------------------------------------------------------- the program

def _build(meta, weights):
    Wc, C, T, S = meta["Wc"], meta["C"], meta["T"], meta["S"]
    NTCH, BKP, TT, F = meta["NTCH"], meta["BKP"], meta["TT"], meta["F"]
    NMH, NMT = meta["NMH"], meta["NMT"]
    has_bt2 = meta["has_bt2"]
    has_bc2 = meta["has_bc2"]

    nc = bacc.Bacc(None)
    dp = nc.declare_dram_parameter
    hx_d = dp("hx", [NMH, P, 1024], F32, isOutput=False)
    slot_d = dp("slot", [P, C], F32, isOutput=False)
    rinv_d = dp("rinv", [P, C], F32, isOutput=False)
    pre_d = dp("pre", [S // 1024, D, 1024], F32, isOutput=False)
    tx_d = dp("tx", [NMT, P, 1024], F32, isOutput=False)
    trel_d = dp("trel", [1, BKP], F32, isOutput=False)
    wd = {}
    for name, arr in weights.items():
        wd[name] = dp(name, list(arr.shape), F32, isOutput=False)
    out_d = dp("logits", [1, BKP], F32, isOutput=True)

    relu = mybir.ActivationFunctionType.Relu
    fcopy = mybir.ActivationFunctionType.Copy
    A = mybir.AluOpType

    with tile.TileContext(nc) as tc:
        import contextlib
        with contextlib.ExitStack() as ctx:
            const = ctx.enter_context(tc.tile_pool(name="const", bufs=1))
            big = ctx.enter_context(tc.tile_pool(name="big", bufs=1))
            hxp = ctx.enter_context(tc.tile_pool(name="hxp", bufs=2))
            sb1 = ctx.enter_context(tc.tile_pool(name="sb1", bufs=2))
            hrmp = ctx.enter_context(tc.tile_pool(name="hrmp", bufs=2))
            segp = ctx.enter_context(tc.tile_pool(name="segp", bufs=2))
            prep = ctx.enter_context(tc.tile_pool(name="prep", bufs=2))
            cwp = ctx.enter_context(tc.tile_pool(name="cwp", bufs=2))
            pbig = ctx.enter_context(
                tc.tile_pool(name="pbig", bufs=3, space="PSUM"))

            def load_const(name, shape):
                t = const.tile(list(shape), F32, tag=name)
                nc.sync.dma_start(out=t[:], in_=wd[name][:])
                return t
            wt1d = load_const("wt1d", (P, D))
            bt1 = load_const("b_t1", (P, 1))
            wt2 = load_const("W_t2", (D, D))
            bt2 = load_const("b_t2", (P, 1))
            wc1b = load_const("W_c1b", (D, D))
            wc2 = load_const("W_c2", (D, D))
            wh1a = load_const("W_h1a", (D, D))
            wh1b = load_const("W_h1b", (D, D))
            bh1 = load_const("b_h1", (P, 1))
            wh2 = load_const("W_h2", (D, 1))
            iota4 = load_const("iota4", (P, 512))
            iota_c = load_const("iota_c", (P, 1))
            ones_r = load_const("ones_r", (1, P))
            b2bc = load_const("b2bc", (P, 512)) if has_bt2 else None
            bc2bc = load_const("bc2bc", (P, 512)) if has_bc2 else None

            slot_sb = const.tile([P, C], F32, tag="slot")
            nc.sync.dma_start(out=slot_sb[:], in_=slot_d[:])
            rinv_sb = const.tile([P, C], F32, tag="rinv")
            nc.sync.dma_start(out=rinv_sb[:], in_=rinv_d[:])

            card_h = big.tile([P, (Wc + 2) * P], F32, tag="card_h")
            tgt_h = big.tile([P, BKP], F32, tag="tgt_h")
            nc.vector.memset(card_h[:, Wc * P:], 0.0)

            # ---------------- card-MLP group (windows 4g..4g+3)
            def card_group(g, pseg_t, pre_mac):
                aggT = cwp.tile([P, 512], F32, tag="aggT")
                nc.vector.tensor_copy(out=aggT[:], in_=pseg_t[:])
                pc1 = pbig.tile([P, 512], F32, tag="pb")
                nc.tensor.matmul(out=pc1[:], lhsT=wc1b[:], rhs=aggT[:],
                                 start=True, stop=True)
                c1 = cwp.tile([P, 512], F32, tag="c1")
                nc.vector.tensor_tensor(
                    out=c1[:], in0=pc1[:],
                    in1=pre_mac[:, (g % 2) * 512:(g % 2) * 512 + 512],
                    op=A.add)
                nc.scalar.activation(c1[:], c1[:], relu)
                prm2 = pbig.tile([P, 512], F32, tag="pb")
                for j in range(4):
                    nc.tensor.matmul(
                        out=prm2[:, j * P:(j + 1) * P],
                        lhsT=c1[:, j * P:(j + 1) * P], rhs=wc2[:],
                        start=True, stop=True)
                dst = card_h[:, 4 * g * P:(4 * g + 4) * P]
                if has_bc2:
                    nc.vector.tensor_tensor(out=dst, in0=prm2[:],
                                            in1=bc2bc[:], op=A.add)
                    nc.vector.tensor_scalar_max(out=dst, in0=dst, scalar1=0.0)
                else:
                    nc.vector.tensor_scalar_max(out=dst, in0=prm2[:],
                                                scalar1=0.0)

            # ---------------- hist loop
            with tc.tile_pool(name="pseg", bufs=2, space="PSUM") as pseg:
                pseg_t = None
                pre_mac = None
                for t in range(T):
                    j = t % MACT
                    if j == 0:
                        hx_mac = hxp.tile([P, 1024], F32, tag="hx")
                        nc.sync.dma_start(out=hx_mac[:], in_=hx_d[t // MACT])
                    ph = (j % 2) * 64
                    cc = (j // 2) * 512
                    ph1 = pbig.tile([P, 512], F32, tag="pb")
                    nc.tensor.matmul(out=ph1[:],
                                     lhsT=wt1d[ph:ph + 64, :],
                                     rhs=hx_mac[ph:ph + 64, cc:cc + 512],
                                     start=True, stop=True)
                    h1 = sb1.tile([P, 512], F32, tag="h1")
                    nc.scalar.activation(h1[:], ph1[:], relu, bias=bt1[:, 0:1])
                    prm = pbig.tile([P, 512], F32, tag="pb")
                    for kk in range(4):
                        nc.tensor.matmul(
                            out=prm[:, kk * P:(kk + 1) * P],
                            lhsT=h1[:, kk * P:(kk + 1) * P], rhs=wt2[:],
                            start=True, stop=True)
                    hrm = hrmp.tile([P, 512], F32, tag="hrm")
                    seg4 = segp.tile([P, 512], F32, tag="seg")
                    nc.vector.tensor_tensor(
                        out=seg4[:].rearrange("p (a b) -> p a b", b=P),
                        in0=slot_sb[:, 4 * t:4 * t + 4].to_broadcast([P, 4, P]),
                        in1=iota4[:].rearrange("p (a b) -> p a b", b=P),
                        op=A.is_equal)
                    if has_bt2:
                        nc.vector.tensor_tensor(out=hrm[:], in0=prm[:],
                                                in1=b2bc[:], op=A.add)
                    for kk in range(4):
                        chunk = 4 * t + kk
                        w, ci = divmod(chunk, CPW)
                        qs = slice(kk * P, (kk + 1) * P)
                        nc.vector.tensor_scalar(
                            out=hrm[:, qs],
                            in0=(hrm[:, qs] if has_bt2 else prm[:, qs]),
                            scalar1=0.0,
                            scalar2=rinv_sb[:, chunk:chunk + 1],
                            op0=A.max, op1=A.mult)
                        if ci == 0 and w % 4 == 0:
                            pseg_t = pseg.tile([P, 512], F32, tag="ps")
                        nc.tensor.matmul(
                            out=pseg_t[:, (w % 4) * P:(w % 4 + 1) * P],
                            lhsT=hrm[:, qs], rhs=seg4[:, qs],
                            start=(ci == 0), stop=(ci == CPW - 1))
                        if ci == CPW - 1 and w % 4 == 3:
                            g = w // 4
                            if g % 2 == 0:
                                pre_mac = prep.tile([P, 1024], F32, tag="pre")
                                nc.sync.dma_start(out=pre_mac[:],
                                                  in_=pre_d[g // 2])
                            card_group(g, pseg_t, pre_mac)

            # ---------------- target MLP
            for t in range(TT):
                j = t % MACT
                if j == 0:
                    tx_mac = hxp.tile([P, 1024], F32, tag="hx")
                    nc.sync.dma_start(out=tx_mac[:], in_=tx_d[t // MACT])
                ph = (j % 2) * 64
                cc = (j // 2) * 512
                ph1 = pbig.tile([P, 512], F32, tag="pb")
                nc.tensor.matmul(out=ph1[:], lhsT=wt1d[ph:ph + 64, :],
                                 rhs=tx_mac[ph:ph + 64, cc:cc + 512],
                                 start=True, stop=True)
                h1 = sb1.tile([P, 512], F32, tag="h1")
                nc.scalar.activation(h1[:], ph1[:], relu, bias=bt1[:, 0:1])
                ph2 = pbig.tile([P, 512], F32, tag="pb")
                nc.tensor.matmul(out=ph2[:], lhsT=wt2[:], rhs=h1[:],
                                 start=True, stop=True)
                nc.scalar.activation(tgt_h[:, t * 512:(t + 1) * 512], ph2[:],
                                     relu, bias=bt2[:, 0:1])

            # ---------------- gather + head
            with (tc.tile_pool(name="pgat", bufs=2, space="PSUM") as pgat,
                  tc.tile_pool(name="plog", bufs=2, space="PSUM") as plog,
                  tc.tile_pool(name="headp", bufs=2) as headp,
                  tc.tile_pool(name="selp", bufs=4) as selp):
                for g in range(TT):
                    trg = headp.tile([1, 512], F32, tag="trg")
                    nc.sync.dma_start(
                        out=trg[:], in_=trel_d[0:1, g * 512:(g + 1) * 512])
                    ptb = pbig.tile([P, 512], F32, tag="pb")
                    nc.tensor.matmul(
                        out=ptb[:], lhsT=ones_r[:], rhs=trg[:],
                        start=True, stop=True)
                    tb = headp.tile([P, 512], F32, tag="tb")
                    nc.vector.tensor_copy(out=tb[:], in_=ptb[:])
                    tch = headp.tile([P, 512], F32, tag="tch")
                    for kk in range(4):
                        c = 4 * g + kk
                        pg = pgat.tile([P, P], F32, tag="pg")
                        for jj in range(TGT_J):
                            sel = selp.tile([P, P], F32, tag="sel")
                            nc.vector.tensor_scalar(
                                out=sel[:],
                                in0=tb[:, kk * P:(kk + 1) * P],
                                scalar1=iota_c[:, 0:1],
                                scalar2=float(P * jj),
                                op0=A.subtract,
                                op1=A.is_equal)
                            nc.tensor.matmul(
                                out=pg[:],
                                lhsT=card_h[:, (2 * c + jj) * P:(2 * c + jj + 1) * P],
                                rhs=sel[:],
                                start=(jj == 0), stop=(jj == TGT_J - 1))
                        nc.vector.tensor_copy(
                            out=tch[:, kk * P:(kk + 1) * P], in_=pg[:])
                    phh = pbig.tile([P, 512], F32, tag="pb")
                    nc.tensor.matmul(out=phh[:], lhsT=wh1a[:],
                                     rhs=tgt_h[:, g * 512:(g + 1) * 512],
                                     start=True, stop=False)
                    nc.tensor.matmul(out=phh[:], lhsT=wh1b[:], rhs=tch[:],
                                     start=False, stop=True)
                    hh = headp.tile([P, 512], F32, tag="hh")
                    nc.scalar.activation(hh[:], phh[:], relu, bias=bh1[:, 0:1])
                    pl = plog.tile([1, 512], F32, tag="pl")
                    nc.tensor.matmul(out=pl[:], lhsT=wh2[:], rhs=hh[:],
                                     start=True, stop=True)
                    ol = headp.tile([1, 512], F32, tag="ol")
                    nc.scalar.activation(ol[:], pl[:], fcopy,
                                         bias=float(weights["b_h2"][0, 0]))
                    nc.sync.dma_start(
                        out=out_d[0:1, g * 512:(g + 1) * 512], in_=ol[:])
    nc.finalize()
    return nc


# ------------------------------------------------------------------ driver

def _weights_dict(W_t1, b_t1, W_t2, b_t2, W_c1, b_c1, W_c2, b_c2,
                  W_h1, b_h1, W_h2, b_h2, CF):
    col = lambda v: np.ascontiguousarray(v.reshape(-1, 1).astype(np.float32))
    iota = np.arange(P, dtype=np.float32)
    w = dict(
        wt1d=np.ascontiguousarray(
            np.vstack([W_t1, W_t1]).astype(np.float32)),
        b_t1=col(b_t1),
        W_t2=np.ascontiguousarray(W_t2, np.float32),
        b_t2=col(b_t2),
        W_c1b=np.ascontiguousarray(W_c1[CF:], np.float32),
        W_c2=np.ascontiguousarray(W_c2, np.float32),
        W_h1a=np.ascontiguousarray(W_h1[:D], np.float32),
        W_h1b=np.ascontiguousarray(W_h1[D:], np.float32),
        b_h1=col(b_h1),
        W_h2=np.ascontiguousarray(W_h2, np.float32),
        b_h2=np.array([[float(np.asarray(b_h2).reshape(-1)[0])]], np.float32),
        iota4=np.ascontiguousarray(np.tile(iota[None, :], (P, 4))),
        iota_c=col(iota),
        ones_r=np.ones((1, P), np.float32),
    )
    has_bt2 = bool(np.any(np.asarray(b_t2) != 0))
    has_bc2 = bool(np.any(np.asarray(b_c2) != 0))
    if has_bt2:
        w["b2bc"] = np.ascontiguousarray(
            np.tile(np.asarray(b_t2, np.float32).reshape(1, -1), (P, 4)))
    if has_bc2:
        w["bc2bc"] = np.ascontiguousarray(
            np.tile(np.asarray(b_c2, np.float32).reshape(1, -1), (P, 4)))
    return w, has_bt2, has_bc2


def _prepare(inputs):
    gi = lambda n: np.asarray(inputs[n])
    target_x = gi("target_x").astype(np.float32)
    hist_x = gi("hist_x").astype(np.float32)
    hist_idx = gi("hist_card_local_idx").astype(np.int64)
    target_idx = gi("target_card_local_idx").astype(np.int64)
    card_dense = gi("card_dense_feats").astype(np.float32)
    CF = card_dense.shape[1]
    weights, has_bt2, has_bc2 = _weights_dict(
        gi("W_t1"), gi("b_t1"), gi("W_t2"), gi("b_t2"),
        gi("W_c1"), gi("b_c1"), gi("W_c2"), gi("b_c2"),
        gi("W_h1"), gi("b_h1"), gi("W_h2"), gi("b_h2"), CF)
    in_maps, scatter, meta = _plan(
        target_x, hist_x, hist_idx, target_idx, card_dense,
        gi("W_c1").astype(np.float32), gi("b_c1").astype(np.float32))
    meta["has_bt2"] = has_bt2
    meta["has_bc2"] = has_bc2
    for m in in_maps:
        m.update(weights)
    return in_maps, scatter, meta, weights, target_x.shape[0]


def kernel(**inputs):
    in_maps, scatter, meta, weights, B = _prepare(inputs)
    nc = _build(meta, weights)

    if os.environ.get("BASS_KERNEL_SIM"):
        from concourse import bass_interp
        outs = []
        for k in range(NCORES):
            sim = bass_interp.CoreSim(nc)
            for name, arr in in_maps[k].items():
                sim.tensor(name)[:] = arr
            sim.simulate()
            outs.append(np.array(sim.tensor("logits")))
    else:
        from concourse.bass_utils import run_bass_kernel_spmd
        res = run_bass_kernel_spmd(nc, in_maps, list(range(NCORES)))
        outs = [res.results[k]["logits"] for k in range(NCORES)]

    out = np.zeros(B, np.float32)
    for k in range(NCORES):
        ids, pos = scatter[k]
        out[ids] = outs[k].reshape(-1)[pos]
    return out


# revision 13
# speedup vs baseline: 1.4289x; 1.2809x over previous
"""CardHistorySAGE Trainium2 kernel (8-core SPMD, no collectives).

Strategy
--------
Host: sort hist rows by card id, walk cards into fixed-shape "windows"
(<= RPW rows, <= 128 cards each, rows padded to exactly RPW).  Consecutive
windows are dealt to the 8 cores in equal blocks, so every card's history
rows AND every target referencing that card land on the same core: the
per-card segment-mean, card MLP, gather and head are all core-local.

All data-dependent decisions (row permutation, in-window card slots,
1/count row scales, target->chunk packing) are baked into *input
tensors*; the instruction stream is identical on all 8 cores.

Device (per core), all fp32:
  hist MLP   : mm1 feature-major (K=64 row-half packing), mm2 row-major
               (lhsT = h1 chunks) -> row-major h in PSUM, fused
               relu*inv_count on DVE
  segment    : one-hot SegMat (batched DVE is_equal); seg matmul
               lhsT=h_rows rhs=SegMat accumulates feature-major per-card
               sums for 4 windows per PSUM bank
  card MLP   : mm1 feature-major + host-precomputed dense-feat term,
               mm2 row-major -> row-major card_h table in SBUF
  gather     : selection-matrix matmuls pull targets' card vectors
  head       : feature-major 2-layer head -> logits
Host: scatter per-core logits back to the original target order.
"""

import os
import numpy as np

from concourse import bass, bacc, mybir
import concourse.tile as tile

F32 = mybir.dt.float32
BF16 = mybir.dt.bfloat16

P = 128          # partitions / chunk rows
D = 128          # hidden dim
CPW = 7          # chunks per window
RPW = CPW * P    # rows per window (896)
TGT_J = 4        # card-windows covered by one target chunk (512 slots)
TSTRIDE = 256    # slot stride between target chunks (2 windows)
NCORES = 8
MACT = 4         # hist/target 512-row tiles per DMA macro


def _pack_tiles(tiles):
    """[N, 64, 512] f32 -> [ceil(N/4), 128, 1024]; tile t at partition-half
    (t%4)%2, column-chunk (t%4)//2 of macro t//4."""
    N = tiles.shape[0]
    NM = -(-N // MACT)
    out = np.zeros((NM, 128, 1024), tiles.dtype)
    for t in range(N):
        j = t % MACT
        out[t // MACT, (j % 2) * 64:(j % 2) * 64 + 64,
            (j // 2) * 512:(j // 2) * 512 + 512] = tiles[t]
    return out


# ----------------------------------------------------------------- planning

def _plan(target_x, hist_x, hist_idx, target_idx, card_dense,
          W_c1, b_c1):
    """Builds per-core input tensors + schedule metadata."""
    B, F = target_x.shape
    U, CF = card_dense.shape

    counts = np.bincount(hist_idx, minlength=U).astype(np.int64)
    assert counts.max() <= RPW, "single card exceeds one window"
    order = np.argsort(hist_idx, kind="stable")
    sidx = hist_idx[order]
    cum = np.concatenate([[0], np.cumsum(counts)])  # rows before card c

    # walk cards into windows
    w_c0 = []
    w_c1_ = []
    c0 = 0
    while c0 < U:
        hi = np.searchsorted(cum, cum[c0] + RPW, side="right") - 1
        c1 = min(hi, c0 + P, U)
        assert c1 > c0
        w_c0.append(c0)
        w_c1_.append(c1)
        c0 = c1
    nwin = len(w_c0)
    w_c0 = np.array(w_c0)
    w_c1_ = np.array(w_c1_)

    Wpc = -(-nwin // NCORES)          # real windows dealt per core
    Wc = -(-(Wpc + 1) // 8) * 8       # +1: local window 0 is a reserved pad
    assert Wc - 1 >= Wpc
    C = Wc * CPW                      # chunks per core
    T = C * P // 512                  # hist 512-row tiles per core
    S = Wc * P                        # card slots per core
    NTCH = Wc // 2                    # target chunks per core
    BKP = NTCH * P                    # padded targets per core
    TT = BKP // 512                   # target 512 tiles / head groups

    win_of_card = np.zeros(U, np.int64)
    slot_of_card = np.zeros(U, np.int64)
    for w in range(nwin):
        win_of_card[w_c0[w]:w_c1_[w]] = w
        slot_of_card[w_c0[w]:w_c1_[w]] = np.arange(w_c1_[w] - w_c0[w])

    in_maps = []
    scatter = []   # (orig_ids, positions) per core
    for k in range(NCORES):
        hx_pad = np.zeros((Wc * RPW, F), np.float32)
        slot_pad = np.full(Wc * RPW, -1.0, np.float32)
        rinv_pad = np.zeros(Wc * RPW, np.float32)
        cdf_slot = np.zeros((S, CF), np.float32)
        for wl in range(1, Wc):
            w = k * Wpc + (wl - 1)
            if w >= min((k + 1) * Wpc, nwin):
                continue
            a, b = w_c0[w], w_c1_[w]
            r0, r1 = cum[a], cum[b]
            n = r1 - r0
            hx_pad[wl * RPW: wl * RPW + n] = hist_x[order[r0:r1]]
            slot_pad[wl * RPW: wl * RPW + n] = (sidx[r0:r1] - a).astype(np.float32)
            rinv_pad[wl * RPW: wl * RPW + n] = (
                1.0 / counts[sidx[r0:r1]]).astype(np.float32)
            cdf_slot[wl * P: wl * P + (b - a)] = card_dense[a:b]
        import ml_dtypes
        hx_tiles = _pack_tiles(hx_pad.reshape(T, 512, F).transpose(0, 2, 1)
                               .astype(ml_dtypes.bfloat16))
        slot2d = np.ascontiguousarray(slot_pad.reshape(C, P).T)
        rinv2d = np.ascontiguousarray(rinv_pad.reshape(C, P).T)
        pre = (cdf_slot @ W_c1[:CF] + b_c1).astype(np.float32)  # [S, D]
        pre_tiles = np.ascontiguousarray(
            pre.T.reshape(D, S // 1024, 1024).transpose(1, 0, 2))

        # ---- targets of this core
        tw = win_of_card[target_idx]
        ids = np.nonzero((tw >= k * Wpc) & (tw < min((k + 1) * Wpc, nwin)))[0]
        slots_abs = (tw[ids] - k * Wpc + 1) * P + slot_of_card[target_idx[ids]]
        t_ord = np.argsort(slots_abs, kind="stable")
        ids = ids[t_ord]
        slots_abs = slots_abs[t_ord]
        # EDF greedy: earliest feasible chunk (chunk c covers [256c, 256c+512))
        used = np.zeros(NTCH, np.int32)
        chunk_of = np.empty(len(ids), np.int64)
        for i, s in enumerate(slots_abs):
            lo = max(0, (s - TGT_J * P + TSTRIDE) // TSTRIDE)
            hi = min(s // TSTRIDE, NTCH - 1)
            c = lo
            while c <= hi and used[c] >= P:
                c += 1
            assert c <= hi, "target chunk packing failed"
            chunk_of[i] = c
            used[c] += 1
        pos_in = np.zeros(len(ids), np.int64)
        fill = np.zeros(NTCH, np.int64)
        for i in range(len(ids)):
            c = chunk_of[i]
            pos_in[i] = c * P + fill[c]
            fill[c] += 1
        tx_pad = np.zeros((BKP, F), np.float32)
        trel = np.full((1, BKP), -999.0, np.float32)
        tx_pad[pos_in] = target_x[ids]
        trel[0, pos_in] = (slots_abs - TSTRIDE * chunk_of).astype(np.float32)
        tx_tiles = _pack_tiles(tx_pad.reshape(TT, 512, F).transpose(0, 2, 1))

        in_maps.append(dict(
            hx=hx_tiles, slot=slot2d, rinv=rinv2d, pre=pre_tiles,
            tx=tx_tiles, trel=trel,
        ))
        scatter.append((ids, pos_in))

    meta = dict(Wc=Wc, C=C, T=T, S=S, NTCH=NTCH, BKP=BKP, TT=TT, F=F, CF=CF,
                NMH=hx_tiles.shape[0], NMT=tx_tiles.shape[0])
    return in_maps, scatter, meta


# ------------------------------------------------------------- the program

def _build(meta, weights):
    Wc, C, T, S = meta["Wc"], meta["C"], meta["T"], meta["S"]
    NTCH, BKP, TT, F = meta["NTCH"], meta["BKP"], meta["TT"], meta["F"]
    NMH, NMT = meta["NMH"], meta["NMT"]
    has_bt2 = meta["has_bt2"]
    has_bc2 = meta["has_bc2"]

    nc = bacc.Bacc(None)
    dp = nc.declare_dram_parameter
    hx_d = dp("hx", [NMH, P, 1024], BF16, isOutput=False)
    slot_d = dp("slot", [P, C], F32, isOutput=False)
    rinv_d = dp("rinv", [P, C], F32, isOutput=False)
    pre_d = dp("pre", [S // 1024, D, 1024], F32, isOutput=False)
    tx_d = dp("tx", [NMT, P, 1024], F32, isOutput=False)
    trel_d = dp("trel", [1, BKP], F32, isOutput=False)
    import ml_dtypes
    wd = {}
    for name, arr in weights.items():
        dt = BF16 if arr.dtype == ml_dtypes.bfloat16 else F32
        wd[name] = dp(name, list(arr.shape), dt, isOutput=False)
    out_d = dp("logits", [1, BKP], F32, isOutput=True)

    relu = mybir.ActivationFunctionType.Relu
    fcopy = mybir.ActivationFunctionType.Copy
    A = mybir.AluOpType

    with tile.TileContext(nc) as tc:
        import contextlib
        with contextlib.ExitStack() as ctx:
            const = ctx.enter_context(tc.tile_pool(name="const", bufs=1))
            big = ctx.enter_context(tc.tile_pool(name="big", bufs=1))
            hxp = ctx.enter_context(tc.tile_pool(name="hxp", bufs=2))
            sb1 = ctx.enter_context(tc.tile_pool(name="sb1", bufs=2))
            hrmp = ctx.enter_context(tc.tile_pool(name="hrmp", bufs=2))
            segp = ctx.enter_context(tc.tile_pool(name="segp", bufs=2))
            prep = ctx.enter_context(tc.tile_pool(name="prep", bufs=2))
            cwp = ctx.enter_context(tc.tile_pool(name="cwp", bufs=2))
            pbig = ctx.enter_context(
                tc.tile_pool(name="pbig", bufs=3, space="PSUM"))

            def load_const(name, shape, dt=F32):
                t = const.tile(list(shape), dt, tag=name)
                nc.sync.dma_start(out=t[:], in_=wd[name][:])
                return t
            wt1d = load_const("wt1d", (P, D))
            wt1b = load_const("wt1b", (P, D), BF16)
            wt2b = load_const("wt2b", (D, D), BF16)
            bt1 = load_const("b_t1", (P, 1))
            wt2 = load_const("W_t2", (D, D))
            bt2 = load_const("b_t2", (P, 1))
            wc1b = load_const("W_c1b", (D, D))
            wc2 = load_const("W_c2", (D, D))
            wh1a = load_const("W_h1a", (D, D))
            wh1b = load_const("W_h1b", (D, D))
            bh1 = load_const("b_h1", (P, 1))
            wh2 = load_const("W_h2", (D, 1))
            iota4 = load_const("iota4", (P, 512))
            iota_c = load_const("iota_c", (P, 1))
            ones_r = load_const("ones_r", (1, P))
            b2bc = load_const("b2bc", (P, 512)) if has_bt2 else None
            bc2bc = load_const("bc2bc", (P, 512)) if has_bc2 else None

            slot_sb = const.tile([P, C], F32, tag="slot")
            nc.sync.dma_start(out=slot_sb[:], in_=slot_d[:])
            rinv_sb = const.tile([P, C], F32, tag="rinv")
            nc.sync.dma_start(out=rinv_sb[:], in_=rinv_d[:])

            card_h = big.tile([P, (Wc + 2) * P], F32, tag="card_h")
            tgt_h = big.tile([P, BKP], F32, tag="tgt_h")
            nc.vector.memset(card_h[:, Wc * P:], 0.0)

            # ---------------- card-MLP group (windows 4g..4g+3)
            def card_group(g, pseg_t, pre_mac):
                aggT = cwp.tile([P, 512], F32, tag="aggT")
                nc.vector.tensor_copy(out=aggT[:], in_=pseg_t[:])
                pc1 = pbig.tile([P, 512], F32, tag="pb")
                nc.tensor.matmul(out=pc1[:], lhsT=wc1b[:], rhs=aggT[:],
                                 start=True, stop=True)
                c1 = cwp.tile([P, 512], F32, tag="c1")
                nc.vector.tensor_tensor(
                    out=c1[:], in0=pc1[:],
                    in1=pre_mac[:, (g % 2) * 512:(g % 2) * 512 + 512],
                    op=A.add)
                nc.scalar.activation(c1[:], c1[:], relu)
                prm2 = pbig.tile([P, 512], F32, tag="pb")
                for j in range(4):
                    nc.tensor.matmul(
                        out=prm2[:, j * P:(j + 1) * P],
                        lhsT=c1[:, j * P:(j + 1) * P], rhs=wc2[:],
                        start=True, stop=True)
                dst = card_h[:, 4 * g * P:(4 * g + 4) * P]
                if has_bc2:
                    nc.vector.tensor_tensor(out=dst, in0=prm2[:],
                                            in1=bc2bc[:], op=A.add)
                    nc.vector.tensor_scalar_max(out=dst, in0=dst, scalar1=0.0)
                else:
                    nc.vector.tensor_scalar_max(out=dst, in0=prm2[:],
                                                scalar1=0.0)

            # ---------------- hist loop
            with tc.tile_pool(name="pseg", bufs=2, space="PSUM") as pseg:
                pseg_t = None
                pre_mac = None
                for t in range(T):
                    j = t % MACT
                    if j == 0:
                        hx_mac = hxp.tile([P, 1024], BF16, tag="hx")
                        nc.sync.dma_start(out=hx_mac[:], in_=hx_d[t // MACT])
                    ph = (j % 2) * 64
                    cc = (j // 2) * 512
                    ph1 = pbig.tile([P, 512], F32, tag="pb")
                    nc.tensor.matmul(out=ph1[:],
                                     lhsT=wt1b[ph:ph + 64, :],
                                     rhs=hx_mac[ph:ph + 64, cc:cc + 512],
                                     start=True, stop=True)
                    h1 = sb1.tile([P, 512], BF16, tag="h1")
                    nc.scalar.activation(h1[:], ph1[:], relu, bias=bt1[:, 0:1])
                    prm = pbig.tile([P, 512], F32, tag="pb")
                    for kk in range(4):
                        nc.tensor.matmul(
                            out=prm[:, kk * P:(kk + 1) * P],
                            lhsT=h1[:, kk * P:(kk + 1) * P], rhs=wt2b[:],
                            start=True, stop=True)
                    hrm = hrmp.tile([P, 512], BF16, tag="hrm")
                    seg4 = segp.tile([P, 512], BF16, tag="seg")
                    nc.vector.tensor_tensor(
                        out=seg4[:].rearrange("p (a b) -> p a b", b=P),
                        in0=slot_sb[:, 4 * t:4 * t + 4].to_broadcast([P, 4, P]),
                        in1=iota4[:].rearrange("p (a b) -> p a b", b=P),
                        op=A.is_equal)
                    if has_bt2:
                        nc.vector.tensor_tensor(out=hrm[:], in0=prm[:],
                                                in1=b2bc[:], op=A.add)
                    for kk in range(4):
                        chunk = 4 * t + kk
                        w, ci = divmod(chunk, CPW)
                        qs = slice(kk * P, (kk + 1) * P)
                        nc.vector.tensor_scalar(
                            out=hrm[:, qs],
                            in0=(hrm[:, qs] if has_bt2 else prm[:, qs]),
                            scalar1=0.0,
                            scalar2=rinv_sb[:, chunk:chunk + 1],
                            op0=A.max, op1=A.mult)
                        if ci == 0 and w % 4 == 0:
                            pseg_t = pseg.tile([P, 512], F32, tag="ps")
                        nc.tensor.matmul(
                            out=pseg_t[:, (w % 4) * P:(w % 4 + 1) * P],
                            lhsT=hrm[:, qs], rhs=seg4[:, qs],
                            start=(ci == 0), stop=(ci == CPW - 1))
                        if ci == CPW - 1 and w % 4 == 3:
                            g = w // 4
                            if g % 2 == 0:
                                pre_mac = prep.tile([P, 1024], F32, tag="pre")
                                nc.sync.dma_start(out=pre_mac[:],
                                                  in_=pre_d[g // 2])
                            card_group(g, pseg_t, pre_mac)

            # ---------------- target MLP
            for t in range(TT):
                j = t % MACT
                if j == 0:
                    tx_mac = hxp.tile([P, 1024], F32, tag="hx")
                    nc.sync.dma_start(out=tx_mac[:], in_=tx_d[t // MACT])
                ph = (j % 2) * 64
                cc = (j // 2) * 512
                ph1 = pbig.tile([P, 512], F32, tag="pb")
                nc.tensor.matmul(out=ph1[:], lhsT=wt1d[ph:ph + 64, :],
                                 rhs=tx_mac[ph:ph + 64, cc:cc + 512],
                                 start=True, stop=True)
                h1 = sb1.tile([P, 512], F32, tag="h1")
                nc.scalar.activation(h1[:], ph1[:], relu, bias=bt1[:, 0:1])
                ph2 = pbig.tile([P, 512], F32, tag="pb")
                nc.tensor.matmul(out=ph2[:], lhsT=wt2[:], rhs=h1[:],
                                 start=True, stop=True)
                nc.scalar.activation(tgt_h[:, t * 512:(t + 1) * 512], ph2[:],
                                     relu, bias=bt2[:, 0:1])

            # ---------------- gather + head
            with (tc.tile_pool(name="pgat", bufs=2, space="PSUM") as pgat,
                  tc.tile_pool(name="plog", bufs=2, space="PSUM") as plog,
                  tc.tile_pool(name="headp", bufs=2) as headp,
                  tc.tile_pool(name="selp", bufs=4) as selp):
                for g in range(TT):
                    trg = headp.tile([1, 512], F32, tag="trg")
                    nc.sync.dma_start(
                        out=trg[:], in_=trel_d[0:1, g * 512:(g + 1) * 512])
                    ptb = pbig.tile([P, 512], F32, tag="pb")
                    nc.tensor.matmul(
                        out=ptb[:], lhsT=ones_r[:], rhs=trg[:],
                        start=True, stop=True)
                    tb = headp.tile([P, 512], F32, tag="tb")
                    nc.vector.tensor_copy(out=tb[:], in_=ptb[:])
                    tch = headp.tile([P, 512], F32, tag="tch")
                    for kk in range(4):
                        c = 4 * g + kk
                        pg = pgat.tile([P, P], F32, tag="pg")
                        for jj in range(TGT_J):
                            sel = selp.tile([P, P], F32, tag="sel")
                            nc.vector.tensor_scalar(
                                out=sel[:],
                                in0=tb[:, kk * P:(kk + 1) * P],
                                scalar1=iota_c[:, 0:1],
                                scalar2=float(P * jj),
                                op0=A.subtract,
                                op1=A.is_equal)
                            nc.tensor.matmul(
                                out=pg[:],
                                lhsT=card_h[:, (2 * c + jj) * P:(2 * c + jj + 1) * P],
                                rhs=sel[:],
                                start=(jj == 0), stop=(jj == TGT_J - 1))
                        nc.vector.tensor_copy(
                            out=tch[:, kk * P:(kk + 1) * P], in_=pg[:])
                    phh = pbig.tile([P, 512], F32, tag="pb")
                    nc.tensor.matmul(out=phh[:], lhsT=wh1a[:],
                                     rhs=tgt_h[:, g * 512:(g + 1) * 512],
                                     start=True, stop=False)
                    nc.tensor.matmul(out=phh[:], lhsT=wh1b[:], rhs=tch[:],
                                     start=False, stop=True)
                    hh = headp.tile([P, 512], F32, tag="hh")
                    nc.scalar.activation(hh[:], phh[:], relu, bias=bh1[:, 0:1])
                    pl = plog.tile([1, 512], F32, tag="pl")
                    nc.tensor.matmul(out=pl[:], lhsT=wh2[:], rhs=hh[:],
                                     start=True, stop=True)
                    ol = headp.tile([1, 512], F32, tag="ol")
                    nc.scalar.activation(ol[:], pl[:], fcopy,
                                         bias=float(weights["b_h2"][0, 0]))
                    nc.sync.dma_start(
                        out=out_d[0:1, g * 512:(g + 1) * 512], in_=ol[:])
    nc.finalize()
    return nc


# ------------------------------------------------------------------ driver

def _weights_dict(W_t1, b_t1, W_t2, b_t2, W_c1, b_c1, W_c2, b_c2,
                  W_h1, b_h1, W_h2, b_h2, CF):
    import ml_dtypes
    col = lambda v: np.ascontiguousarray(v.reshape(-1, 1).astype(np.float32))
    iota = np.arange(P, dtype=np.float32)
    w = dict(
        wt1d=np.ascontiguousarray(
            np.vstack([W_t1, W_t1]).astype(np.float32)),
        wt1b=np.ascontiguousarray(
            np.vstack([W_t1, W_t1]).astype(ml_dtypes.bfloat16)),
        wt2b=np.ascontiguousarray(
            np.asarray(W_t2).astype(ml_dtypes.bfloat16)),
        b_t1=col(b_t1),
        W_t2=np.ascontiguousarray(W_t2, np.float32),
        b_t2=col(b_t2),
        W_c1b=np.ascontiguousarray(W_c1[CF:], np.float32),
        W_c2=np.ascontiguousarray(W_c2, np.float32),
        W_h1a=np.ascontiguousarray(W_h1[:D], np.float32),
        W_h1b=np.ascontiguousarray(W_h1[D:], np.float32),
        b_h1=col(b_h1),
        W_h2=np.ascontiguousarray(W_h2, np.float32),
        b_h2=np.array([[float(np.asarray(b_h2).reshape(-1)[0])]], np.float32),
        iota4=np.ascontiguousarray(np.tile(iota[None, :], (P, 4))),
        iota_c=col(iota),
        ones_r=np.ones((1, P), np.float32),
    )
    has_bt2 = bool(np.any(np.asarray(b_t2) != 0))
    has_bc2 = bool(np.any(np.asarray(b_c2) != 0))
    if has_bt2:
        w["b2bc"] = np.ascontiguousarray(
            np.tile(np.asarray(b_t2, np.float32).reshape(1, -1), (P, 4)))
    if has_bc2:
        w["bc2bc"] = np.ascontiguousarray(
            np.tile(np.asarray(b_c2, np.float32).reshape(1, -1), (P, 4)))
    return w, has_bt2, has_bc2


def _prepare(inputs):
    gi = lambda n: np.asarray(inputs[n])
    target_x = gi("target_x").astype(np.float32)
    hist_x = gi("hist_x").astype(np.float32)
    hist_idx = gi("hist_card_local_idx").astype(np.int64)
    target_idx = gi("target_card_local_idx").astype(np.int64)
    card_dense = gi("card_dense_feats").astype(np.float32)
    CF = card_dense.shape[1]
    weights, has_bt2, has_bc2 = _weights_dict(
        gi("W_t1"), gi("b_t1"), gi("W_t2"), gi("b_t2"),
        gi("W_c1"), gi("b_c1"), gi("W_c2"), gi("b_c2"),
        gi("W_h1"), gi("b_h1"), gi("W_h2"), gi("b_h2"), CF)
    in_maps, scatter, meta = _plan(
        target_x, hist_x, hist_idx, target_idx, card_dense,
        gi("W_c1").astype(np.float32), gi("b_c1").astype(np.float32))
    meta["has_bt2"] = has_bt2
    meta["has_bc2"] = has_bc2
    for m in in_maps:
        m.update(weights)
    return in_maps, scatter, meta, weights, target_x.shape[0]


def kernel(**inputs):
    in_maps, scatter, meta, weights, B = _prepare(inputs)
    nc = _build(meta, weights)

    if os.environ.get("BASS_KERNEL_SIM"):
        from concourse import bass_interp
        outs = []
        for k in range(NCORES):
            sim = bass_interp.CoreSim(nc)
            for name, arr in in_maps[k].items():
                sim.tensor(name)[:] = arr
            sim.simulate()
            outs.append(np.array(sim.tensor("logits")))
    else:
        from concourse.bass_utils import run_bass_kernel_spmd
        res = run_bass_kernel_spmd(nc, in_maps, list(range(NCORES)))
        outs = [res.results[k]["logits"] for k in range(NCORES)]

    out = np.zeros(B, np.float32)
    for k in range(NCORES):
        ids, pos = scatter[k]
        out[ids] = outs[k].reshape(-1)[pos]
    return out


# revision 14
# speedup vs baseline: 1.6118x; 1.1280x over previous
"""CardHistorySAGE Trainium2 kernel (8-core SPMD, no collectives).

Strategy
--------
Host: sort hist rows by card id, walk cards into fixed-shape "windows"
(<= RPW rows, <= 128 cards each, rows padded to exactly RPW).  Consecutive
windows are dealt to the 8 cores in equal blocks, so every card's history
rows AND every target referencing that card land on the same core: the
per-card segment-mean, card MLP, gather and head are all core-local.

All data-dependent decisions (row permutation, in-window card slots,
1/count row scales, target->chunk packing) are baked into *input
tensors*; the instruction stream is identical on all 8 cores.

Device (per core), all fp32:
  hist MLP   : mm1 feature-major (K=64 row-half packing), mm2 row-major
               (lhsT = h1 chunks) -> row-major h in PSUM, fused
               relu*inv_count on DVE
  segment    : one-hot SegMat (batched DVE is_equal); seg matmul
               lhsT=h_rows rhs=SegMat accumulates feature-major per-card
               sums for 4 windows per PSUM bank
  card MLP   : mm1 feature-major + host-precomputed dense-feat term,
               mm2 row-major -> row-major card_h table in SBUF
  gather     : selection-matrix matmuls pull targets' card vectors
  head       : feature-major 2-layer head -> logits
Host: scatter per-core logits back to the original target order.
"""

import os
import numpy as np

from concourse import bass, bacc, mybir
import concourse.tile as tile

F32 = mybir.dt.float32
BF16 = mybir.dt.bfloat16

P = 128          # partitions / chunk rows
D = 128          # hidden dim
CPW = 7          # chunks per window
RPW = CPW * P    # rows per window (896)
TGT_J = 4        # card-windows covered by one target chunk (512 slots)
TSTRIDE = 256    # slot stride between target chunks (2 windows)
NCORES = 8
MACT = 4         # hist/target 512-row tiles per DMA macro


def _pack_tiles(tiles):
    """[N, 64, 512] f32 -> [ceil(N/4), 128, 1024]; tile t at partition-half
    (t%4)%2, column-chunk (t%4)//2 of macro t//4."""
    N = tiles.shape[0]
    NM = -(-N // MACT)
    out = np.zeros((NM, 128, 1024), tiles.dtype)
    for t in range(N):
        j = t % MACT
        out[t // MACT, (j % 2) * 64:(j % 2) * 64 + 64,
            (j // 2) * 512:(j // 2) * 512 + 512] = tiles[t]
    return out


# ----------------------------------------------------------------- planning

def _plan(target_x, hist_x, hist_idx, target_idx, card_dense,
          W_c1, b_c1):
    """Builds per-core input tensors + schedule metadata."""
    B, F = target_x.shape
    U, CF = card_dense.shape

    counts = np.bincount(hist_idx, minlength=U).astype(np.int64)
    assert counts.max() <= RPW, "single card exceeds one window"
    order = np.argsort(hist_idx, kind="stable")
    sidx = hist_idx[order]
    cum = np.concatenate([[0], np.cumsum(counts)])  # rows before card c

    # walk cards into windows
    w_c0 = []
    w_c1_ = []
    c0 = 0
    while c0 < U:
        hi = np.searchsorted(cum, cum[c0] + RPW, side="right") - 1
        c1 = min(hi, c0 + P, U)
        assert c1 > c0
        w_c0.append(c0)
        w_c1_.append(c1)
        c0 = c1
    nwin = len(w_c0)
    w_c0 = np.array(w_c0)
    w_c1_ = np.array(w_c1_)

    Wpc = -(-nwin // NCORES)          # real windows dealt per core
    Wc = -(-(Wpc + 1) // 8) * 8       # +1: local window 0 is a reserved pad
    assert Wc - 1 >= Wpc
    C = Wc * CPW                      # chunks per core
    T = C * P // 512                  # hist 512-row tiles per core
    S = Wc * P                        # card slots per core
    NTCH = Wc // 2                    # target chunks per core
    BKP = NTCH * P                    # padded targets per core
    TT = BKP // 512                   # target 512 tiles / head groups

    win_of_card = np.zeros(U, np.int64)
    slot_of_card = np.zeros(U, np.int64)
    for w in range(nwin):
        win_of_card[w_c0[w]:w_c1_[w]] = w
        slot_of_card[w_c0[w]:w_c1_[w]] = np.arange(w_c1_[w] - w_c0[w])

    in_maps = []
    scatter = []   # (orig_ids, positions) per core
    for k in range(NCORES):
        hx_pad = np.zeros((Wc * RPW, F), np.float32)
        slot_pad = np.full(Wc * RPW, -1.0, np.float32)
        cnt_slot = np.zeros(S, np.float32)
        cdf_slot = np.zeros((S, CF), np.float32)
        for wl in range(1, Wc):
            w = k * Wpc + (wl - 1)
            if w >= min((k + 1) * Wpc, nwin):
                continue
            a, b = w_c0[w], w_c1_[w]
            r0, r1 = cum[a], cum[b]
            n = r1 - r0
            hx_pad[wl * RPW: wl * RPW + n] = hist_x[order[r0:r1]]
            slot_pad[wl * RPW: wl * RPW + n] = (sidx[r0:r1] - a).astype(np.float32)
            cnt_slot[wl * P: wl * P + (b - a)] = counts[a:b]
            cdf_slot[wl * P: wl * P + (b - a)] = card_dense[a:b]
        import ml_dtypes
        hx_tiles = _pack_tiles(hx_pad.reshape(T, 512, F).transpose(0, 2, 1)
                               .astype(ml_dtypes.bfloat16))
        slot2d = np.ascontiguousarray(slot_pad.reshape(C, P).T)
        invr = np.ascontiguousarray(
            (1.0 / np.maximum(cnt_slot, 1.0)).reshape(S // 512, 1, 512))
        pre = (cdf_slot @ W_c1[:CF] + b_c1).astype(np.float32)  # [S, D]
        pre_tiles = np.ascontiguousarray(
            pre.T.reshape(D, S // 1024, 1024).transpose(1, 0, 2))

        # ---- targets of this core
        tw = win_of_card[target_idx]
        ids = np.nonzero((tw >= k * Wpc) & (tw < min((k + 1) * Wpc, nwin)))[0]
        slots_abs = (tw[ids] - k * Wpc + 1) * P + slot_of_card[target_idx[ids]]
        t_ord = np.argsort(slots_abs, kind="stable")
        ids = ids[t_ord]
        slots_abs = slots_abs[t_ord]
        # EDF greedy: earliest feasible chunk (chunk c covers [256c, 256c+512))
        used = np.zeros(NTCH, np.int32)
        chunk_of = np.empty(len(ids), np.int64)
        for i, s in enumerate(slots_abs):
            lo = max(0, (s - TGT_J * P + TSTRIDE) // TSTRIDE)
            hi = min(s // TSTRIDE, NTCH - 1)
            c = lo
            while c <= hi and used[c] >= P:
                c += 1
            assert c <= hi, "target chunk packing failed"
            chunk_of[i] = c
            used[c] += 1
        pos_in = np.zeros(len(ids), np.int64)
        fill = np.zeros(NTCH, np.int64)
        for i in range(len(ids)):
            c = chunk_of[i]
            pos_in[i] = c * P + fill[c]
            fill[c] += 1
        tx_pad = np.zeros((BKP, F), np.float32)
        trel = np.full((1, BKP), -999.0, np.float32)
        tx_pad[pos_in] = target_x[ids]
        trel[0, pos_in] = (slots_abs - TSTRIDE * chunk_of).astype(np.float32)
        tx_tiles = _pack_tiles(tx_pad.reshape(TT, 512, F).transpose(0, 2, 1))

        in_maps.append(dict(
            hx=hx_tiles, slot=slot2d, invr=invr, pre=pre_tiles,
            tx=tx_tiles, trel=trel,
        ))
        scatter.append((ids, pos_in))

    meta = dict(Wc=Wc, C=C, T=T, S=S, NTCH=NTCH, BKP=BKP, TT=TT, F=F, CF=CF,
                NMH=hx_tiles.shape[0], NMT=tx_tiles.shape[0])
    return in_maps, scatter, meta


# ------------------------------------------------------------- the program

def _build(meta, weights):
    Wc, C, T, S = meta["Wc"], meta["C"], meta["T"], meta["S"]
    NTCH, BKP, TT, F = meta["NTCH"], meta["BKP"], meta["TT"], meta["F"]
    NMH, NMT = meta["NMH"], meta["NMT"]
    has_bt2 = meta["has_bt2"]
    has_bc2 = meta["has_bc2"]

    nc = bacc.Bacc(None)
    dp = nc.declare_dram_parameter
    hx_d = dp("hx", [NMH, P, 1024], BF16, isOutput=False)
    slot_d = dp("slot", [P, C], F32, isOutput=False)
    invr_d = dp("invr", [S // 512, 1, 512], F32, isOutput=False)
    pre_d = dp("pre", [S // 1024, D, 1024], F32, isOutput=False)
    tx_d = dp("tx", [NMT, P, 1024], F32, isOutput=False)
    trel_d = dp("trel", [1, BKP], F32, isOutput=False)
    import ml_dtypes
    wd = {}
    for name, arr in weights.items():
        dt = BF16 if arr.dtype == ml_dtypes.bfloat16 else F32
        wd[name] = dp(name, list(arr.shape), dt, isOutput=False)
    out_d = dp("logits", [1, BKP], F32, isOutput=True)

    relu = mybir.ActivationFunctionType.Relu
    fcopy = mybir.ActivationFunctionType.Copy
    A = mybir.AluOpType

    with tile.TileContext(nc) as tc:
        import contextlib
        with contextlib.ExitStack() as ctx:
            const = ctx.enter_context(tc.tile_pool(name="const", bufs=1))
            big = ctx.enter_context(tc.tile_pool(name="big", bufs=1))
            hxp = ctx.enter_context(tc.tile_pool(name="hxp", bufs=2))
            sb1 = ctx.enter_context(tc.tile_pool(name="sb1", bufs=2))
            hrmp = ctx.enter_context(tc.tile_pool(name="hrmp", bufs=2))
            segp = ctx.enter_context(tc.tile_pool(name="segp", bufs=2))
            prep = ctx.enter_context(tc.tile_pool(name="prep", bufs=2))
            cwp = ctx.enter_context(tc.tile_pool(name="cwp", bufs=2))
            pbig = ctx.enter_context(
                tc.tile_pool(name="pbig", bufs=3, space="PSUM"))

            def load_const(name, shape, dt=F32):
                t = const.tile(list(shape), dt, tag=name)
                nc.sync.dma_start(out=t[:], in_=wd[name][:])
                return t
            wt1d = load_const("wt1d", (P, D))
            wt1b = load_const("wt1b", (P, D), BF16)
            wt2b = load_const("wt2b", (D, D), BF16)
            bt1 = load_const("b_t1", (P, 1))
            wt2 = load_const("W_t2", (D, D))
            bt2 = load_const("b_t2", (P, 1))
            wc1b = load_const("W_c1b", (D, D))
            wc2 = load_const("W_c2", (D, D))
            wh1a = load_const("W_h1a", (D, D))
            wh1b = load_const("W_h1b", (D, D))
            bh1 = load_const("b_h1", (P, 1))
            wh2 = load_const("W_h2", (D, 1))
            iota4 = load_const("iota4", (P, 512))
            iota_c = load_const("iota_c", (P, 1))
            b2bc = load_const("b2bc", (P, 512)) if has_bt2 else None
            bc2bc = load_const("bc2bc", (P, 512)) if has_bc2 else None

            slot_sb = const.tile([P, C], F32, tag="slot")
            nc.sync.dma_start(out=slot_sb[:], in_=slot_d[:])
            invp = ctx.enter_context(tc.tile_pool(name="invp", bufs=2))

            card_h = big.tile([P, (Wc + 2) * P], BF16, tag="card_h")
            tgt_h = big.tile([P, BKP], F32, tag="tgt_h")
            nc.vector.memset(card_h[:, Wc * P:], 0.0)

            # ---------------- card-MLP group (windows 4g..4g+3)
            def card_group(g, pseg_t, pre_mac):
                iv1 = invp.tile([1, 512], F32, tag="iv1")
                nc.sync.dma_start(out=iv1[:], in_=invr_d[g])
                ivb = invp.tile([P, 512], F32, tag="ivb")
                nc.gpsimd.partition_broadcast(ivb[:], iv1[:])
                aggT = cwp.tile([P, 512], F32, tag="aggT")
                nc.any.tensor_tensor(out=aggT[:], in0=pseg_t[:], in1=ivb[:],
                                     op=A.mult)
                pc1 = pbig.tile([P, 512], F32, tag="pb")
                nc.tensor.matmul(out=pc1[:], lhsT=wc1b[:], rhs=aggT[:],
                                 start=True, stop=True)
                c1 = cwp.tile([P, 512], F32, tag="c1")
                nc.vector.tensor_tensor(
                    out=c1[:], in0=pc1[:],
                    in1=pre_mac[:, (g % 2) * 512:(g % 2) * 512 + 512],
                    op=A.add)
                nc.scalar.activation(c1[:], c1[:], relu)
                prm2 = pbig.tile([P, 512], F32, tag="pb")
                for j in range(4):
                    nc.tensor.matmul(
                        out=prm2[:, j * P:(j + 1) * P],
                        lhsT=c1[:, j * P:(j + 1) * P], rhs=wc2[:],
                        start=True, stop=True)
                dst = card_h[:, 4 * g * P:(4 * g + 4) * P]
                if has_bc2:
                    nc.any.tensor_tensor(out=dst, in0=prm2[:],
                                         in1=bc2bc[:], op=A.add)
                    nc.any.tensor_scalar_max(out=dst, in0=dst, scalar1=0.0)
                else:
                    nc.any.tensor_scalar_max(out=dst, in0=prm2[:],
                                             scalar1=0.0)

            # ---------------- hist loop
            with tc.tile_pool(name="pseg", bufs=2, space="PSUM") as pseg:
                pseg_t = None
                pre_mac = None
                for t in range(T):
                    j = t % MACT
                    if j == 0:
                        hx_mac = hxp.tile([P, 1024], BF16, tag="hx")
                        nc.sync.dma_start(out=hx_mac[:], in_=hx_d[t // MACT])
                    ph = (j % 2) * 64
                    cc = (j // 2) * 512
                    ph1 = pbig.tile([P, 512], F32, tag="pb")
                    nc.tensor.matmul(out=ph1[:],
                                     lhsT=wt1b[ph:ph + 64, :],
                                     rhs=hx_mac[ph:ph + 64, cc:cc + 512],
                                     start=True, stop=True)
                    h1 = sb1.tile([P, 512], BF16, tag="h1")
                    nc.scalar.activation(h1[:], ph1[:], relu, bias=bt1[:, 0:1])
                    prm = pbig.tile([P, 512], F32, tag="pb")
                    for kk in range(4):
                        nc.tensor.matmul(
                            out=prm[:, kk * P:(kk + 1) * P],
                            lhsT=h1[:, kk * P:(kk + 1) * P], rhs=wt2b[:],
                            start=True, stop=True)
                    hrm = hrmp.tile([P, 512], BF16, tag="hrm")
                    seg4 = segp.tile([P, 512], BF16, tag="seg")
                    nc.any.tensor_tensor(
                        out=seg4[:].rearrange("p (a b) -> p a b", b=P),
                        in0=slot_sb[:, 4 * t:4 * t + 4].to_broadcast([P, 4, P]),
                        in1=iota4[:].rearrange("p (a b) -> p a b", b=P),
                        op=A.is_equal)
                    if has_bt2:
                        nc.any.tensor_tensor(out=hrm[:], in0=prm[:],
                                             in1=b2bc[:], op=A.add)
                        nc.any.tensor_scalar_max(out=hrm[:], in0=hrm[:],
                                                 scalar1=0.0)
                    else:
                        nc.any.tensor_scalar_max(out=hrm[:], in0=prm[:],
                                                 scalar1=0.0)
                    for kk in range(4):
                        chunk = 4 * t + kk
                        w, ci = divmod(chunk, CPW)
                        qs = slice(kk * P, (kk + 1) * P)
                        if ci == 0 and w % 4 == 0:
                            pseg_t = pseg.tile([P, 512], F32, tag="ps")
                        nc.tensor.matmul(
                            out=pseg_t[:, (w % 4) * P:(w % 4 + 1) * P],
                            lhsT=hrm[:, qs], rhs=seg4[:, qs],
                            start=(ci == 0), stop=(ci == CPW - 1))
                        if ci == CPW - 1 and w % 4 == 3:
                            g = w // 4
                            if g % 2 == 0:
                                pre_mac = prep.tile([P, 1024], F32, tag="pre")
                                nc.sync.dma_start(out=pre_mac[:],
                                                  in_=pre_d[g // 2])
                            card_group(g, pseg_t, pre_mac)

            # ---------------- target MLP
            for t in range(TT):
                j = t % MACT
                if j == 0:
                    tx_mac = hxp.tile([P, 1024], F32, tag="hx")
                    nc.sync.dma_start(out=tx_mac[:], in_=tx_d[t // MACT])
                ph = (j % 2) * 64
                cc = (j // 2) * 512
                ph1 = pbig.tile([P, 512], F32, tag="pb")
                nc.tensor.matmul(out=ph1[:], lhsT=wt1d[ph:ph + 64, :],
                                 rhs=tx_mac[ph:ph + 64, cc:cc + 512],
                                 start=True, stop=True)
                h1 = sb1.tile([P, 512], F32, tag="h1")
                nc.scalar.activation(h1[:], ph1[:], relu, bias=bt1[:, 0:1])
                ph2 = pbig.tile([P, 512], F32, tag="pb")
                nc.tensor.matmul(out=ph2[:], lhsT=wt2[:], rhs=h1[:],
                                 start=True, stop=True)
                nc.scalar.activation(tgt_h[:, t * 512:(t + 1) * 512], ph2[:],
                                     relu, bias=bt2[:, 0:1])

            # ---------------- gather + head
            with (tc.tile_pool(name="pgat", bufs=2, space="PSUM") as pgat,
                  tc.tile_pool(name="plog", bufs=2, space="PSUM") as plog,
                  tc.tile_pool(name="headp", bufs=2) as headp,
                  tc.tile_pool(name="selp", bufs=4) as selp):
                for g in range(TT):
                    trg = headp.tile([1, 512], F32, tag="trg")
                    nc.sync.dma_start(
                        out=trg[:], in_=trel_d[0:1, g * 512:(g + 1) * 512])
                    tb = headp.tile([P, 512], F32, tag="tb")
                    nc.gpsimd.partition_broadcast(tb[:], trg[:])
                    tch = headp.tile([P, 512], F32, tag="tch")
                    for kk in range(4):
                        c = 4 * g + kk
                        pg = pgat.tile([P, P], F32, tag="pg")
                        for jj in range(TGT_J):
                            sel = selp.tile([P, P], BF16, tag="sel")
                            nc.any.tensor_scalar(
                                out=sel[:],
                                in0=tb[:, kk * P:(kk + 1) * P],
                                scalar1=iota_c[:, 0:1],
                                scalar2=float(P * jj),
                                op0=A.subtract,
                                op1=A.is_equal)
                            nc.tensor.matmul(
                                out=pg[:],
                                lhsT=card_h[:, (2 * c + jj) * P:(2 * c + jj + 1) * P],
                                rhs=sel[:],
                                start=(jj == 0), stop=(jj == TGT_J - 1))
                        nc.any.tensor_copy(
                            out=tch[:, kk * P:(kk + 1) * P], in_=pg[:])
                    phh = pbig.tile([P, 512], F32, tag="pb")
                    nc.tensor.matmul(out=phh[:], lhsT=wh1a[:],
                                     rhs=tgt_h[:, g * 512:(g + 1) * 512],
                                     start=True, stop=False)
                    nc.tensor.matmul(out=phh[:], lhsT=wh1b[:], rhs=tch[:],
                                     start=False, stop=True)
                    hh = headp.tile([P, 512], F32, tag="hh")
                    nc.scalar.activation(hh[:], phh[:], relu, bias=bh1[:, 0:1])
                    pl = plog.tile([1, 512], F32, tag="pl")
                    nc.tensor.matmul(out=pl[:], lhsT=wh2[:], rhs=hh[:],
                                     start=True, stop=True)
                    ol = headp.tile([1, 512], F32, tag="ol")
                    nc.scalar.activation(ol[:], pl[:], fcopy,
                                         bias=float(weights["b_h2"][0, 0]))
                    nc.sync.dma_start(
                        out=out_d[0:1, g * 512:(g + 1) * 512], in_=ol[:])
    nc.finalize()
    return nc


# ------------------------------------------------------------------ driver

def _weights_dict(W_t1, b_t1, W_t2, b_t2, W_c1, b_c1, W_c2, b_c2,
                  W_h1, b_h1, W_h2, b_h2, CF):
    import ml_dtypes
    col = lambda v: np.ascontiguousarray(v.reshape(-1, 1).astype(np.float32))
    iota = np.arange(P, dtype=np.float32)
    w = dict(
        wt1d=np.ascontiguousarray(
            np.vstack([W_t1, W_t1]).astype(np.float32)),
        wt1b=np.ascontiguousarray(
            np.vstack([W_t1, W_t1]).astype(ml_dtypes.bfloat16)),
        wt2b=np.ascontiguousarray(
            np.asarray(W_t2).astype(ml_dtypes.bfloat16)),
        b_t1=col(b_t1),
        W_t2=np.ascontiguousarray(W_t2, np.float32),
        b_t2=col(b_t2),
        W_c1b=np.ascontiguousarray(W_c1[CF:], np.float32),
        W_c2=np.ascontiguousarray(W_c2, np.float32),
        W_h1a=np.ascontiguousarray(W_h1[:D], np.float32),
        W_h1b=np.ascontiguousarray(W_h1[D:], np.float32),
        b_h1=col(b_h1),
        W_h2=np.ascontiguousarray(W_h2, np.float32),
        b_h2=np.array([[float(np.asarray(b_h2).reshape(-1)[0])]], np.float32),
        iota4=np.ascontiguousarray(np.tile(iota[None, :], (P, 4))),
        iota_c=col(iota),
    )
    has_bt2 = bool(np.any(np.asarray(b_t2) != 0))
    has_bc2 = bool(np.any(np.asarray(b_c2) != 0))
    if has_bt2:
        w["b2bc"] = np.ascontiguousarray(
            np.tile(np.asarray(b_t2, np.float32).reshape(1, -1), (P, 4)))
    if has_bc2:
        w["bc2bc"] = np.ascontiguousarray(
            np.tile(np.asarray(b_c2, np.float32).reshape(1, -1), (P, 4)))
    return w, has_bt2, has_bc2


def _prepare(inputs):
    gi = lambda n: np.asarray(inputs[n])
    target_x = gi("target_x").astype(np.float32)
    hist_x = gi("hist_x").astype(np.float32)
    hist_idx = gi("hist_card_local_idx").astype(np.int64)
    target_idx = gi("target_card_local_idx").astype(np.int64)
    card_dense = gi("card_dense_feats").astype(np.float32)
    CF = card_dense.shape[1]
    weights, has_bt2, has_bc2 = _weights_dict(
        gi("W_t1"), gi("b_t1"), gi("W_t2"), gi("b_t2"),
        gi("W_c1"), gi("b_c1"), gi("W_c2"), gi("b_c2"),
        gi("W_h1"), gi("b_h1"), gi("W_h2"), gi("b_h2"), CF)
    in_maps, scatter, meta = _plan(
        target_x, hist_x, hist_idx, target_idx, card_dense,
        gi("W_c1").astype(np.float32), gi("b_c1").astype(np.float32))
    meta["has_bt2"] = has_bt2
    meta["has_bc2"] = has_bc2
    for m in in_maps:
        m.update(weights)
    return in_maps, scatter, meta, weights, target_x.shape[0]


def kernel(**inputs):
    in_maps, scatter, meta, weights, B = _prepare(inputs)
    nc = _build(meta, weights)

    if os.environ.get("BASS_KERNEL_SIM"):
        from concourse import bass_interp
        outs = []
        for k in range(NCORES):
            sim = bass_interp.CoreSim(nc)
            for name, arr in in_maps[k].items():
                sim.tensor(name)[:] = arr
            sim.simulate()
            outs.append(np.array(sim.tensor("logits")))
    else:
        from concourse.bass_utils import run_bass_kernel_spmd
        res = run_bass_kernel_spmd(nc, in_maps, list(range(NCORES)))
        outs = [res.results[k]["logits"] for k in range(NCORES)]

    out = np.zeros(B, np.float32)
    for k in range(NCORES):
        ids, pos = scatter[k]
        out[ids] = outs[k].reshape(-1)[pos]
    return out


# revision 15
# speedup vs baseline: 1.6369x; 1.0155x over previous
"""CardHistorySAGE Trainium2 kernel (8-core SPMD, no collectives).

Strategy
--------
Host: sort hist rows by card id, walk cards into fixed-shape "windows"
(<= RPW rows, <= 128 cards each, rows padded to exactly RPW).  Consecutive
windows are dealt to the 8 cores in equal blocks, so every card's history
rows AND every target referencing that card land on the same core: the
per-card segment-mean, card MLP, gather and head are all core-local.

All data-dependent decisions (row permutation, in-window card slots,
1/count row scales, target->chunk packing) are baked into *input
tensors*; the instruction stream is identical on all 8 cores.

Device (per core), all fp32:
  hist MLP   : mm1 feature-major (K=64 row-half packing), mm2 row-major
               (lhsT = h1 chunks) -> row-major h in PSUM, fused
               relu*inv_count on DVE
  segment    : one-hot SegMat (batched DVE is_equal); seg matmul
               lhsT=h_rows rhs=SegMat accumulates feature-major per-card
               sums for 4 windows per PSUM bank
  card MLP   : mm1 feature-major + host-precomputed dense-feat term,
               mm2 row-major -> row-major card_h table in SBUF
  gather     : selection-matrix matmuls pull targets' card vectors
  head       : feature-major 2-layer head -> logits
Host: scatter per-core logits back to the original target order.
"""

import os
import numpy as np

from concourse import bass, bacc, mybir
import concourse.tile as tile

F32 = mybir.dt.float32
BF16 = mybir.dt.bfloat16

P = 128          # partitions / chunk rows
D = 128          # hidden dim
CPW = 7          # chunks per window
RPW = CPW * P    # rows per window (896)
TGT_J = 4        # card-windows covered by one target chunk (512 slots)
TSTRIDE = 256    # slot stride between target chunks (2 windows)
NCORES = 8
MACT = 4         # hist/target 512-row tiles per DMA macro


def _pack_tiles(tiles):
    """[N, 64, 512] f32 -> [ceil(N/4), 128, 1024]; tile t at partition-half
    (t%4)%2, column-chunk (t%4)//2 of macro t//4."""
    N = tiles.shape[0]
    NM = -(-N // MACT)
    out = np.zeros((NM, 128, 1024), tiles.dtype)
    for t in range(N):
        j = t % MACT
        out[t // MACT, (j % 2) * 64:(j % 2) * 64 + 64,
            (j // 2) * 512:(j // 2) * 512 + 512] = tiles[t]
    return out


# ----------------------------------------------------------------- planning

def _plan(target_x, hist_x, hist_idx, target_idx, card_dense,
          W_c1, b_c1):
    """Builds per-core input tensors + schedule metadata."""
    B, F = target_x.shape
    U, CF = card_dense.shape

    counts = np.bincount(hist_idx, minlength=U).astype(np.int64)
    assert counts.max() <= RPW, "single card exceeds one window"
    order = np.argsort(hist_idx, kind="stable")
    sidx = hist_idx[order]
    cum = np.concatenate([[0], np.cumsum(counts)])  # rows before card c

    # walk cards into windows
    w_c0 = []
    w_c1_ = []
    c0 = 0
    while c0 < U:
        hi = np.searchsorted(cum, cum[c0] + RPW, side="right") - 1
        c1 = min(hi, c0 + P, U)
        assert c1 > c0
        w_c0.append(c0)
        w_c1_.append(c1)
        c0 = c1
    nwin = len(w_c0)
    w_c0 = np.array(w_c0)
    w_c1_ = np.array(w_c1_)

    Wpc = -(-nwin // NCORES)          # real windows dealt per core
    Wc = -(-(Wpc + 1) // 8) * 8       # +1: local window 0 is a reserved pad
    assert Wc - 1 >= Wpc
    C = Wc * CPW                      # chunks per core
    T = C * P // 512                  # hist 512-row tiles per core
    S = Wc * P                        # card slots per core
    NTCH = Wc // 2                    # target chunks per core
    BKP = NTCH * P                    # padded targets per core
    TT = BKP // 512                   # target 512 tiles / head groups

    win_of_card = np.zeros(U, np.int64)
    slot_of_card = np.zeros(U, np.int64)
    for w in range(nwin):
        win_of_card[w_c0[w]:w_c1_[w]] = w
        slot_of_card[w_c0[w]:w_c1_[w]] = np.arange(w_c1_[w] - w_c0[w])

    in_maps = []
    scatter = []   # (orig_ids, positions) per core
    for k in range(NCORES):
        hx_pad = np.zeros((Wc * RPW, F), np.float32)
        slot_pad = np.full(Wc * RPW, -1.0, np.float32)
        cnt_slot = np.zeros(S, np.float32)
        cdf_slot = np.zeros((S, CF), np.float32)
        for wl in range(1, Wc):
            w = k * Wpc + (wl - 1)
            if w >= min((k + 1) * Wpc, nwin):
                continue
            a, b = w_c0[w], w_c1_[w]
            r0, r1 = cum[a], cum[b]
            n = r1 - r0
            hx_pad[wl * RPW: wl * RPW + n] = hist_x[order[r0:r1]]
            slot_pad[wl * RPW: wl * RPW + n] = (sidx[r0:r1] - a).astype(np.float32)
            cnt_slot[wl * P: wl * P + (b - a)] = counts[a:b]
            cdf_slot[wl * P: wl * P + (b - a)] = card_dense[a:b]
        import ml_dtypes
        hx_tiles = _pack_tiles(hx_pad.reshape(T, 512, F).transpose(0, 2, 1)
                               .astype(ml_dtypes.bfloat16))
        slot2d = np.ascontiguousarray(slot_pad.reshape(C, P).T)
        invr = np.ascontiguousarray(
            (1.0 / np.maximum(cnt_slot, 1.0)).reshape(S // 512, 1, 512))
        pre = (cdf_slot @ W_c1[:CF] + b_c1).astype(np.float32)  # [S, D]
        pre_tiles = np.ascontiguousarray(
            pre.T.reshape(D, S // 1024, 1024).transpose(1, 0, 2))

        # ---- targets of this core
        tw = win_of_card[target_idx]
        ids = np.nonzero((tw >= k * Wpc) & (tw < min((k + 1) * Wpc, nwin)))[0]
        slots_abs = (tw[ids] - k * Wpc + 1) * P + slot_of_card[target_idx[ids]]
        t_ord = np.argsort(slots_abs, kind="stable")
        ids = ids[t_ord]
        slots_abs = slots_abs[t_ord]
        # EDF greedy: earliest feasible chunk (chunk c covers [256c, 256c+512))
        used = np.zeros(NTCH, np.int32)
        chunk_of = np.empty(len(ids), np.int64)
        for i, s in enumerate(slots_abs):
            lo = max(0, (s - TGT_J * P + TSTRIDE) // TSTRIDE)
            hi = min(s // TSTRIDE, NTCH - 1)
            c = lo
            while c <= hi and used[c] >= P:
                c += 1
            assert c <= hi, "target chunk packing failed"
            chunk_of[i] = c
            used[c] += 1
        pos_in = np.zeros(len(ids), np.int64)
        fill = np.zeros(NTCH, np.int64)
        for i in range(len(ids)):
            c = chunk_of[i]
            pos_in[i] = c * P + fill[c]
            fill[c] += 1
        tx_pad = np.zeros((BKP, F), np.float32)
        trel = np.full((1, BKP), -999.0, np.float32)
        tx_pad[pos_in] = target_x[ids]
        trel[0, pos_in] = (slots_abs - TSTRIDE * chunk_of).astype(np.float32)
        tx_tiles = _pack_tiles(tx_pad.reshape(TT, 512, F).transpose(0, 2, 1))

        in_maps.append(dict(
            hx=hx_tiles, slot=slot2d, invr=invr, pre=pre_tiles,
            tx=tx_tiles, trel=trel,
        ))
        scatter.append((ids, pos_in))

    meta = dict(Wc=Wc, C=C, T=T, S=S, NTCH=NTCH, BKP=BKP, TT=TT, F=F, CF=CF,
                NMH=hx_tiles.shape[0], NMT=tx_tiles.shape[0])
    return in_maps, scatter, meta


# ------------------------------------------------------------- the program

def _build(meta, weights):
    Wc, C, T, S = meta["Wc"], meta["C"], meta["T"], meta["S"]
    NTCH, BKP, TT, F = meta["NTCH"], meta["BKP"], meta["TT"], meta["F"]
    NMH, NMT = meta["NMH"], meta["NMT"]
    has_bt2 = meta["has_bt2"]
    has_bc2 = meta["has_bc2"]

    nc = bacc.Bacc(None)
    dp = nc.declare_dram_parameter
    hx_d = dp("hx", [NMH, P, 1024], BF16, isOutput=False)
    slot_d = dp("slot", [P, C], F32, isOutput=False)
    invr_d = dp("invr", [S // 512, 1, 512], F32, isOutput=False)
    pre_d = dp("pre", [S // 1024, D, 1024], F32, isOutput=False)
    tx_d = dp("tx", [NMT, P, 1024], F32, isOutput=False)
    trel_d = dp("trel", [1, BKP], F32, isOutput=False)
    import ml_dtypes
    wd = {}
    for name, arr in weights.items():
        dt = BF16 if arr.dtype == ml_dtypes.bfloat16 else F32
        wd[name] = dp(name, list(arr.shape), dt, isOutput=False)
    out_d = dp("logits", [1, BKP], F32, isOutput=True)

    relu = mybir.ActivationFunctionType.Relu
    fcopy = mybir.ActivationFunctionType.Copy
    A = mybir.AluOpType

    with tile.TileContext(nc) as tc:
        import contextlib
        with contextlib.ExitStack() as ctx:
            const = ctx.enter_context(tc.tile_pool(name="const", bufs=1))
            big = ctx.enter_context(tc.tile_pool(name="big", bufs=1))
            hxp = ctx.enter_context(tc.tile_pool(name="hxp", bufs=2))
            sb1 = ctx.enter_context(tc.tile_pool(name="sb1", bufs=3))
            hrmp = ctx.enter_context(tc.tile_pool(name="hrmp", bufs=3))
            segp = ctx.enter_context(tc.tile_pool(name="segp", bufs=3))
            prep = ctx.enter_context(tc.tile_pool(name="prep", bufs=2))
            cwp = ctx.enter_context(tc.tile_pool(name="cwp", bufs=2))
            pbig = ctx.enter_context(
                tc.tile_pool(name="pbig", bufs=4, space="PSUM"))

            def load_const(name, shape, dt=F32):
                t = const.tile(list(shape), dt, tag=name)
                nc.sync.dma_start(out=t[:], in_=wd[name][:])
                return t
            wt1d = load_const("wt1d", (P, D))
            wt1b = load_const("wt1b", (P, D), BF16)
            wt2b = load_const("wt2b", (D, D), BF16)
            bt1 = load_const("b_t1", (P, 1))
            wt2 = load_const("W_t2", (D, D))
            bt2 = load_const("b_t2", (P, 1))
            wc1b = load_const("W_c1b", (D, D))
            wc2 = load_const("W_c2", (D, D))
            wh1a = load_const("W_h1a", (D, D))
            wh1b = load_const("W_h1b", (D, D))
            bh1 = load_const("b_h1", (P, 1))
            wh2 = load_const("W_h2", (D, 1))
            iota4 = load_const("iota4", (P, 512))
            iota_c = load_const("iota_c", (P, 1))
            b2bc = load_const("b2bc", (P, 512)) if has_bt2 else None
            bc2bc = load_const("bc2bc", (P, 512)) if has_bc2 else None

            slot_sb = const.tile([P, C], F32, tag="slot")
            nc.sync.dma_start(out=slot_sb[:], in_=slot_d[:])
            invp = ctx.enter_context(tc.tile_pool(name="invp", bufs=2))

            card_h = big.tile([P, (Wc + 2) * P], BF16, tag="card_h")
            tgt_h = big.tile([P, BKP], F32, tag="tgt_h")
            nc.vector.memset(card_h[:, Wc * P:], 0.0)

            # ---------------- card-MLP group (windows 4g..4g+3)
            def card_group(g, pseg_t, pre_mac):
                ivb = invp.tile([P, 512], F32, tag="ivb")
                nc.sync.dma_start(out=ivb[:],
                                  in_=invr_d[g].to_broadcast([P, 512]))
                aggT = cwp.tile([P, 512], F32, tag="aggT")
                nc.any.tensor_tensor(out=aggT[:], in0=pseg_t[:], in1=ivb[:],
                                     op=A.mult)
                pc1 = pbig.tile([P, 512], F32, tag="pb")
                nc.tensor.matmul(out=pc1[:], lhsT=wc1b[:], rhs=aggT[:],
                                 start=True, stop=True)
                c1 = cwp.tile([P, 512], F32, tag="c1")
                nc.vector.tensor_tensor(
                    out=c1[:], in0=pc1[:],
                    in1=pre_mac[:, (g % 2) * 512:(g % 2) * 512 + 512],
                    op=A.add)
                nc.scalar.activation(c1[:], c1[:], relu)
                prm2 = pbig.tile([P, 512], F32, tag="pb")
                for j in range(4):
                    nc.tensor.matmul(
                        out=prm2[:, j * P:(j + 1) * P],
                        lhsT=c1[:, j * P:(j + 1) * P], rhs=wc2[:],
                        start=True, stop=True)
                dst = card_h[:, 4 * g * P:(4 * g + 4) * P]
                if has_bc2:
                    nc.any.tensor_tensor(out=dst, in0=prm2[:],
                                         in1=bc2bc[:], op=A.add)
                    nc.any.tensor_scalar_max(out=dst, in0=dst, scalar1=0.0)
                else:
                    nc.any.tensor_scalar_max(out=dst, in0=prm2[:],
                                             scalar1=0.0)

            # ---------------- hist loop
            with tc.tile_pool(name="pseg", bufs=2, space="PSUM") as pseg:
                pseg_t = None
                pre_mac = None
                for t in range(T):
                    j = t % MACT
                    if j == 0:
                        hx_mac = hxp.tile([P, 1024], BF16, tag="hx")
                        nc.sync.dma_start(out=hx_mac[:], in_=hx_d[t // MACT])
                    ph = (j % 2) * 64
                    cc = (j // 2) * 512
                    ph1 = pbig.tile([P, 512], F32, tag="pb")
                    nc.tensor.matmul(out=ph1[:],
                                     lhsT=wt1b[ph:ph + 64, :],
                                     rhs=hx_mac[ph:ph + 64, cc:cc + 512],
                                     start=True, stop=True)
                    h1 = sb1.tile([P, 512], BF16, tag="h1")
                    nc.scalar.activation(h1[:], ph1[:], relu, bias=bt1[:, 0:1])
                    prm = pbig.tile([P, 512], F32, tag="pb")
                    for kk in range(4):
                        nc.tensor.matmul(
                            out=prm[:, kk * P:(kk + 1) * P],
                            lhsT=h1[:, kk * P:(kk + 1) * P], rhs=wt2b[:],
                            start=True, stop=True)
                    hrm = hrmp.tile([P, 512], BF16, tag="hrm")
                    seg4 = segp.tile([P, 512], BF16, tag="seg")
                    nc.any.tensor_tensor(
                        out=seg4[:].rearrange("p (a b) -> p a b", b=P),
                        in0=slot_sb[:, 4 * t:4 * t + 4].to_broadcast([P, 4, P]),
                        in1=iota4[:].rearrange("p (a b) -> p a b", b=P),
                        op=A.is_equal)
                    if has_bt2:
                        nc.any.tensor_tensor(out=hrm[:], in0=prm[:],
                                             in1=b2bc[:], op=A.add)
                        nc.any.tensor_scalar_max(out=hrm[:], in0=hrm[:],
                                                 scalar1=0.0)
                    else:
                        nc.any.tensor_scalar_max(out=hrm[:], in0=prm[:],
                                                 scalar1=0.0)
                    for kk in range(4):
                        chunk = 4 * t + kk
                        w, ci = divmod(chunk, CPW)
                        qs = slice(kk * P, (kk + 1) * P)
                        if ci == 0 and w % 4 == 0:
                            pseg_t = pseg.tile([P, 512], F32, tag="ps")
                        nc.tensor.matmul(
                            out=pseg_t[:, (w % 4) * P:(w % 4 + 1) * P],
                            lhsT=hrm[:, qs], rhs=seg4[:, qs],
                            start=(ci == 0), stop=(ci == CPW - 1))
                        if ci == CPW - 1 and w % 4 == 3:
                            g = w // 4
                            if g % 2 == 0:
                                pre_mac = prep.tile([P, 1024], F32, tag="pre")
                                nc.sync.dma_start(out=pre_mac[:],
                                                  in_=pre_d[g // 2])
                            card_group(g, pseg_t, pre_mac)

            # ---------------- target MLP
            for t in range(TT):
                j = t % MACT
                if j == 0:
                    tx_mac = hxp.tile([P, 1024], F32, tag="hx")
                    nc.sync.dma_start(out=tx_mac[:], in_=tx_d[t // MACT])
                ph = (j % 2) * 64
                cc = (j // 2) * 512
                ph1 = pbig.tile([P, 512], F32, tag="pb")
                nc.tensor.matmul(out=ph1[:], lhsT=wt1d[ph:ph + 64, :],
                                 rhs=tx_mac[ph:ph + 64, cc:cc + 512],
                                 start=True, stop=True)
                h1 = sb1.tile([P, 512], F32, tag="h1")
                nc.scalar.activation(h1[:], ph1[:], relu, bias=bt1[:, 0:1])
                ph2 = pbig.tile([P, 512], F32, tag="pb")
                nc.tensor.matmul(out=ph2[:], lhsT=wt2[:], rhs=h1[:],
                                 start=True, stop=True)
                nc.scalar.activation(tgt_h[:, t * 512:(t + 1) * 512], ph2[:],
                                     relu, bias=bt2[:, 0:1])

            # ---------------- gather + head
            with (tc.tile_pool(name="pgat", bufs=2, space="PSUM") as pgat,
                  tc.tile_pool(name="plog", bufs=2, space="PSUM") as plog,
                  tc.tile_pool(name="headp", bufs=3) as headp,
                  tc.tile_pool(name="selp", bufs=4) as selp):
                for g in range(TT):
                    tb = headp.tile([P, 512], F32, tag="tb")
                    nc.sync.dma_start(
                        out=tb[:],
                        in_=trel_d[0:1, g * 512:(g + 1) * 512]
                        .to_broadcast([P, 512]))
                    tch = headp.tile([P, 512], F32, tag="tch")
                    for kk in range(4):
                        c = 4 * g + kk
                        pg = pgat.tile([P, P], F32, tag="pg")
                        for jj in range(TGT_J):
                            sel = selp.tile([P, P], BF16, tag="sel")
                            nc.any.tensor_scalar(
                                out=sel[:],
                                in0=tb[:, kk * P:(kk + 1) * P],
                                scalar1=iota_c[:, 0:1],
                                scalar2=float(P * jj),
                                op0=A.subtract,
                                op1=A.is_equal)
                            nc.tensor.matmul(
                                out=pg[:],
                                lhsT=card_h[:, (2 * c + jj) * P:(2 * c + jj + 1) * P],
                                rhs=sel[:],
                                start=(jj == 0), stop=(jj == TGT_J - 1))
                        nc.any.tensor_copy(
                            out=tch[:, kk * P:(kk + 1) * P], in_=pg[:])
                    phh = pbig.tile([P, 512], F32, tag="pb")
                    nc.tensor.matmul(out=phh[:], lhsT=wh1a[:],
                                     rhs=tgt_h[:, g * 512:(g + 1) * 512],
                                     start=True, stop=False)
                    nc.tensor.matmul(out=phh[:], lhsT=wh1b[:], rhs=tch[:],
                                     start=False, stop=True)
                    hh = headp.tile([P, 512], F32, tag="hh")
                    nc.scalar.activation(hh[:], phh[:], relu, bias=bh1[:, 0:1])
                    pl = plog.tile([1, 512], F32, tag="pl")
                    nc.tensor.matmul(out=pl[:], lhsT=wh2[:], rhs=hh[:],
                                     start=True, stop=True)
                    ol = headp.tile([1, 512], F32, tag="ol")
                    nc.scalar.activation(ol[:], pl[:], fcopy,
                                         bias=float(weights["b_h2"][0, 0]))
                    nc.sync.dma_start(
                        out=out_d[0:1, g * 512:(g + 1) * 512], in_=ol[:])
    nc.finalize()
    return nc


# ------------------------------------------------------------------ driver

def _weights_dict(W_t1, b_t1, W_t2, b_t2, W_c1, b_c1, W_c2, b_c2,
                  W_h1, b_h1, W_h2, b_h2, CF):
    import ml_dtypes
    col = lambda v: np.ascontiguousarray(v.reshape(-1, 1).astype(np.float32))
    iota = np.arange(P, dtype=np.float32)
    w = dict(
        wt1d=np.ascontiguousarray(
            np.vstack([W_t1, W_t1]).astype(np.float32)),
        wt1b=np.ascontiguousarray(
            np.vstack([W_t1, W_t1]).astype(ml_dtypes.bfloat16)),
        wt2b=np.ascontiguousarray(
            np.asarray(W_t2).astype(ml_dtypes.bfloat16)),
        b_t1=col(b_t1),
        W_t2=np.ascontiguousarray(W_t2, np.float32),
        b_t2=col(b_t2),
        W_c1b=np.ascontiguousarray(W_c1[CF:], np.float32),
        W_c2=np.ascontiguousarray(W_c2, np.float32),
        W_h1a=np.ascontiguousarray(W_h1[:D], np.float32),
        W_h1b=np.ascontiguousarray(W_h1[D:], np.float32),
        b_h1=col(b_h1),
        W_h2=np.ascontiguousarray(W_h2, np.float32),
        b_h2=np.array([[float(np.asarray(b_h2).reshape(-1)[0])]], np.float32),
        iota4=np.ascontiguousarray(np.tile(iota[None, :], (P, 4))),
        iota_c=col(iota),
    )
    has_bt2 = bool(np.any(np.asarray(b_t2) != 0))
    has_bc2 = bool(np.any(np.asarray(b_c2) != 0))
    if has_bt2:
        w["b2bc"] = np.ascontiguousarray(
            np.tile(np.asarray(b_t2, np.float32).reshape(1, -1), (P, 4)))
    if has_bc2:
        w["bc2bc"] = np.ascontiguousarray(
            np.tile(np.asarray(b_c2, np.float32).reshape(1, -1), (P, 4)))
    return w, has_bt2, has_bc2


def _prepare(inputs):
    gi = lambda n: np.asarray(inputs[n])
    target_x = gi("target_x").astype(np.float32)
    hist_x = gi("hist_x").astype(np.float32)
    hist_idx = gi("hist_card_local_idx").astype(np.int64)
    target_idx = gi("target_card_local_idx").astype(np.int64)
    card_dense = gi("card_dense_feats").astype(np.float32)
    CF = card_dense.shape[1]
    weights, has_bt2, has_bc2 = _weights_dict(
        gi("W_t1"), gi("b_t1"), gi("W_t2"), gi("b_t2"),
        gi("W_c1"), gi("b_c1"), gi("W_c2"), gi("b_c2"),
        gi("W_h1"), gi("b_h1"), gi("W_h2"), gi("b_h2"), CF)
    in_maps, scatter, meta = _plan(
        target_x, hist_x, hist_idx, target_idx, card_dense,
        gi("W_c1").astype(np.float32), gi("b_c1").astype(np.float32))
    meta["has_bt2"] = has_bt2
    meta["has_bc2"] = has_bc2
    for m in in_maps:
        m.update(weights)
    return in_maps, scatter, meta, weights, target_x.shape[0]


def kernel(**inputs):
    in_maps, scatter, meta, weights, B = _prepare(inputs)
    nc = _build(meta, weights)

    if os.environ.get("BASS_KERNEL_SIM"):
        from concourse import bass_interp
        outs = []
        for k in range(NCORES):
            sim = bass_interp.CoreSim(nc)
            for name, arr in in_maps[k].items():
                sim.tensor(name)[:] = arr
            sim.simulate()
            outs.append(np.array(sim.tensor("logits")))
    else:
        from concourse.bass_utils import run_bass_kernel_spmd
        res = run_bass_kernel_spmd(nc, in_maps, list(range(NCORES)))
        outs = [res.results[k]["logits"] for k in range(NCORES)]

    out = np.zeros(B, np.float32)
    for k in range(NCORES):
        ids, pos = scatter[k]
        out[ids] = outs[k].reshape(-1)[pos]
    return out


# revision 18
# speedup vs baseline: 3.0807x; 1.8820x over previous
"""CardHistorySAGE Trainium2 kernel (8-core SPMD, no collectives).

Strategy
--------
Host: sort hist rows by card id, walk cards into fixed-shape "windows"
(<= RPW rows, <= 128 cards each, rows padded to exactly RPW).  Consecutive
windows are dealt to the 8 cores in equal blocks, so every card's history
rows AND every target referencing that card land on the same core: the
per-card segment-mean, card MLP, gather and head are all core-local.

All data-dependent decisions (row permutation, in-window card slots,
1/count row scales, target->chunk packing) are baked into *input
tensors*; the instruction stream is identical on all 8 cores.

Device (per core), all fp32:
  hist MLP   : mm1 feature-major (K=64 row-half packing), mm2 row-major
               (lhsT = h1 chunks) -> row-major h in PSUM, fused
               relu*inv_count on DVE
  segment    : one-hot SegMat (batched DVE is_equal); seg matmul
               lhsT=h_rows rhs=SegMat accumulates feature-major per-card
               sums for 4 windows per PSUM bank
  card MLP   : mm1 feature-major + host-precomputed dense-feat term,
               mm2 row-major -> row-major card_h table in SBUF
  gather     : selection-matrix matmuls pull targets' card vectors
  head       : feature-major 2-layer head -> logits
Host: scatter per-core logits back to the original target order.
"""

import os
import numpy as np

from concourse import bass, bacc, mybir
import concourse.tile as tile

F32 = mybir.dt.float32
BF16 = mybir.dt.bfloat16

P = 128          # partitions / chunk rows
D = 128          # hidden dim
CPW = 7          # chunks per window
RPW = CPW * P    # rows per window (896)
TGT_J = 4        # card-windows covered by one target chunk (512 slots)
TSTRIDE = 256    # slot stride between target chunks (2 windows)
NCORES = 8
MACT = 4         # hist/target 512-row tiles per DMA macro


def _pack_tiles(tiles):
    """[N, 64, 512] f32 -> [ceil(N/4), 128, 1024]; tile t at partition-half
    (t%4)%2, column-chunk (t%4)//2 of macro t//4."""
    N = tiles.shape[0]
    NM = -(-N // MACT)
    out = np.zeros((NM, 128, 1024), tiles.dtype)
    for t in range(N):
        j = t % MACT
        out[t // MACT, (j % 2) * 64:(j % 2) * 64 + 64,
            (j // 2) * 512:(j // 2) * 512 + 512] = tiles[t]
    return out


# ----------------------------------------------------------------- planning

def _plan(target_x, hist_x, hist_idx, target_idx, card_dense,
          W_c1, b_c1):
    """Builds per-core input tensors + schedule metadata."""
    B, F = target_x.shape
    U, CF = card_dense.shape

    counts = np.bincount(hist_idx, minlength=U).astype(np.int64)
    assert counts.max() <= RPW, "single card exceeds one window"
    order = np.argsort(hist_idx, kind="stable")
    sidx = hist_idx[order]
    cum = np.concatenate([[0], np.cumsum(counts)])  # rows before card c

    # walk cards into windows
    w_c0 = []
    w_c1_ = []
    c0 = 0
    while c0 < U:
        hi = np.searchsorted(cum, cum[c0] + RPW, side="right") - 1
        c1 = min(hi, c0 + P, U)
        assert c1 > c0
        w_c0.append(c0)
        w_c1_.append(c1)
        c0 = c1
    nwin = len(w_c0)
    w_c0 = np.array(w_c0)
    w_c1_ = np.array(w_c1_)

    Wpc = -(-nwin // NCORES)          # real windows dealt per core
    Wc = -(-(Wpc + 1) // 8) * 8       # +1: local window 0 is a reserved pad
    assert Wc - 1 >= Wpc
    C = Wc * CPW                      # chunks per core
    T = C * P // 512                  # hist 512-row tiles per core
    S = Wc * P                        # card slots per core
    NTCH = Wc // 2                    # target chunks per core
    BKP = NTCH * P                    # padded targets per core
    TT = BKP // 512                   # target 512 tiles / head groups

    win_of_card = np.zeros(U, np.int64)
    slot_of_card = np.zeros(U, np.int64)
    for w in range(nwin):
        win_of_card[w_c0[w]:w_c1_[w]] = w
        slot_of_card[w_c0[w]:w_c1_[w]] = np.arange(w_c1_[w] - w_c0[w])

    in_maps = []
    scatter = []   # (orig_ids, positions) per core
    for k in range(NCORES):
        hx_pad = np.zeros((Wc * RPW, F), np.float32)
        slot_pad = np.full(Wc * RPW, -1.0, np.float32)
        cnt_slot = np.zeros(S, np.float32)
        cdf_slot = np.zeros((S, CF), np.float32)
        for wl in range(1, Wc):
            w = k * Wpc + (wl - 1)
            if w >= min((k + 1) * Wpc, nwin):
                continue
            a, b = w_c0[w], w_c1_[w]
            r0, r1 = cum[a], cum[b]
            n = r1 - r0
            hx_pad[wl * RPW: wl * RPW + n] = hist_x[order[r0:r1]]
            slot_pad[wl * RPW: wl * RPW + n] = (sidx[r0:r1] - a).astype(np.float32)
            cnt_slot[wl * P: wl * P + (b - a)] = counts[a:b]
            cdf_slot[wl * P: wl * P + (b - a)] = card_dense[a:b]
        import ml_dtypes
        hx_tiles = _pack_tiles(hx_pad.reshape(T, 512, F).transpose(0, 2, 1)
                               .astype(ml_dtypes.bfloat16))
        slot2d = np.ascontiguousarray(slot_pad.reshape(C, P).T)
        invr = np.ascontiguousarray(
            (1.0 / np.maximum(cnt_slot, 1.0)).reshape(S // 512, 1, 512))
        pre = (cdf_slot @ W_c1[:CF] + b_c1).astype(np.float32)  # [S, D]
        pre_tiles = np.ascontiguousarray(
            pre.T.reshape(D, S // 1024, 1024).transpose(1, 0, 2))

        # ---- targets of this core
        tw = win_of_card[target_idx]
        ids = np.nonzero((tw >= k * Wpc) & (tw < min((k + 1) * Wpc, nwin)))[0]
        slots_abs = (tw[ids] - k * Wpc + 1) * P + slot_of_card[target_idx[ids]]
        t_ord = np.argsort(slots_abs, kind="stable")
        ids = ids[t_ord]
        slots_abs = slots_abs[t_ord]
        # EDF greedy: earliest feasible chunk (chunk c covers [256c, 256c+512))
        used = np.zeros(NTCH, np.int32)
        chunk_of = np.empty(len(ids), np.int64)
        for i, s in enumerate(slots_abs):
            lo = max(0, (s - TGT_J * P + TSTRIDE) // TSTRIDE)
            hi = min(s // TSTRIDE, NTCH - 1)
            c = lo
            while c <= hi and used[c] >= P:
                c += 1
            assert c <= hi, "target chunk packing failed"
            chunk_of[i] = c
            used[c] += 1
        pos_in = np.zeros(len(ids), np.int64)
        fill = np.zeros(NTCH, np.int64)
        for i in range(len(ids)):
            c = chunk_of[i]
            pos_in[i] = c * P + fill[c]
            fill[c] += 1
        tx_pad = np.zeros((BKP, F), np.float32)
        trel = np.full((1, BKP), -999.0, np.float32)
        tx_pad[pos_in] = target_x[ids]
        trel[0, pos_in] = (slots_abs - TSTRIDE * chunk_of).astype(np.float32)
        tx_tiles = _pack_tiles(tx_pad.reshape(TT, 512, F).transpose(0, 2, 1))

        in_maps.append(dict(
            hx=hx_tiles, slot=slot2d, invr=invr, pre=pre_tiles,
            tx=tx_tiles, trel=trel,
        ))
        scatter.append((ids, pos_in))

    meta = dict(Wc=Wc, C=C, T=T, S=S, NTCH=NTCH, BKP=BKP, TT=TT, F=F, CF=CF,
                NMH=hx_tiles.shape[0], NMT=tx_tiles.shape[0])
    return in_maps, scatter, meta


# ------------------------------------------------------------- the program

def _build(meta, weights):
    Wc, C, T, S = meta["Wc"], meta["C"], meta["T"], meta["S"]
    NTCH, BKP, TT, F = meta["NTCH"], meta["BKP"], meta["TT"], meta["F"]
    NMH, NMT = meta["NMH"], meta["NMT"]
    has_bt2 = meta["has_bt2"]
    has_bc2 = meta["has_bc2"]
    has_bt1 = meta["has_bt1"]
    has_bh1 = meta["has_bh1"]

    nc = bacc.Bacc(None)
    dp = nc.declare_dram_parameter
    hx_d = dp("hx", [NMH, P, 1024], BF16, isOutput=False)
    slot_d = dp("slot", [P, C], F32, isOutput=False)
    invr_d = dp("invr", [S // 512, 1, 512], F32, isOutput=False)
    pre_d = dp("pre", [S // 1024, D, 1024], F32, isOutput=False)
    tx_d = dp("tx", [NMT, P, 1024], F32, isOutput=False)
    trel_d = dp("trel", [1, BKP], F32, isOutput=False)
    import ml_dtypes
    wd = {}
    for name, arr in weights.items():
        dt = BF16 if arr.dtype == ml_dtypes.bfloat16 else F32
        wd[name] = dp(name, list(arr.shape), dt, isOutput=False)
    out_d = dp("logits", [1, BKP], F32, isOutput=True)

    relu = mybir.ActivationFunctionType.Relu
    fcopy = mybir.ActivationFunctionType.Copy
    A = mybir.AluOpType

    with tile.TileContext(nc) as tc:
        import contextlib
        with contextlib.ExitStack() as ctx:
            const = ctx.enter_context(tc.tile_pool(name="const", bufs=1))
            big = ctx.enter_context(tc.tile_pool(name="big", bufs=1))
            hxp = ctx.enter_context(tc.tile_pool(name="hxp", bufs=2))
            sb1 = ctx.enter_context(tc.tile_pool(name="sb1", bufs=3))
            hrmp = ctx.enter_context(tc.tile_pool(name="hrmp", bufs=3))
            segp = ctx.enter_context(tc.tile_pool(name="segp", bufs=3))
            prep = ctx.enter_context(tc.tile_pool(name="prep", bufs=2))
            cwp = ctx.enter_context(tc.tile_pool(name="cwp", bufs=2))
            pbig = ctx.enter_context(
                tc.tile_pool(name="pbig", bufs=2, space="PSUM"))

            def load_const(name, shape, dt=F32):
                t = const.tile(list(shape), dt, tag=name)
                nc.sync.dma_start(out=t[:], in_=wd[name][:])
                return t
            wt1d = load_const("wt1d", (P, D))
            wt1b = load_const("wt1b", (P, D), BF16)
            wt2b = load_const("wt2b", (D, D), BF16)
            bt1 = load_const("b_t1", (P, 1))
            wt2 = load_const("W_t2", (D, D))
            bt2 = load_const("b_t2", (P, 1))
            wc1b = load_const("W_c1b", (D, D))
            wc2 = load_const("W_c2", (D, D))
            wh1a = load_const("W_h1a", (D, D))
            wh1b = load_const("W_h1b", (D, D))
            bh1 = load_const("b_h1", (P, 1))
            wh2 = load_const("W_h2", (D, 1))
            iota4 = load_const("iota4", (P, 512))
            iota_c = load_const("iota_c", (P, 1))
            b2bc = load_const("b2bc", (P, 512)) if has_bt2 else None
            bc2bc = load_const("bc2bc", (P, 512)) if has_bc2 else None

            slot_sb = const.tile([P, C], F32, tag="slot")
            nc.sync.dma_start(out=slot_sb[:], in_=slot_d[:])
            invp = ctx.enter_context(tc.tile_pool(name="invp", bufs=2))

            card_h = big.tile([P, (Wc + 2) * P], BF16, tag="card_h")
            tgt_h = big.tile([P, BKP], F32, tag="tgt_h")
            nc.vector.memset(card_h[:, Wc * P:], 0.0)

            # ---------------- card-MLP group (windows 4g..4g+3)
            def card_group(g, pseg_t, pre_mac):
                ivb = invp.tile([P, 512], F32, tag="ivb")
                nc.sync.dma_start(out=ivb[:],
                                  in_=invr_d[g].to_broadcast([P, 512]))
                aggT = cwp.tile([P, 512], F32, tag="aggT")
                nc.any.tensor_tensor(out=aggT[:], in0=pseg_t[:], in1=ivb[:],
                                     op=A.mult)
                pc1 = pbig.tile([P, 512], F32, tag="pb")
                nc.tensor.matmul(out=pc1[:], lhsT=wc1b[:], rhs=aggT[:],
                                 start=True, stop=True)
                c1 = cwp.tile([P, 512], F32, tag="c1")
                nc.vector.tensor_tensor(
                    out=c1[:], in0=pc1[:],
                    in1=pre_mac[:, (g % 2) * 512:(g % 2) * 512 + 512],
                    op=A.add)
                nc.scalar.activation(c1[:], c1[:], relu)
                prm2 = pbig.tile([P, 512], F32, tag="pb")
                for j in range(4):
                    nc.tensor.matmul(
                        out=prm2[:, j * P:(j + 1) * P],
                        lhsT=c1[:, j * P:(j + 1) * P], rhs=wc2[:],
                        start=True, stop=True)
                dst = card_h[:, 4 * g * P:(4 * g + 4) * P]
                if has_bc2:
                    nc.any.tensor_tensor(out=dst, in0=prm2[:],
                                         in1=bc2bc[:], op=A.add)
                    nc.any.tensor_scalar_max(out=dst, in0=dst, scalar1=0.0)
                else:
                    nc.any.tensor_scalar_max(out=dst, in0=prm2[:],
                                             scalar1=0.0)

            # ---------------- hist loop
            with (tc.tile_pool(name="pseg", bufs=2, space="PSUM") as pseg,
                  tc.tile_pool(name="phist", bufs=2, space="PSUM") as phist):
                pseg_t = None
                pre_mac = None
                for t in range(T):
                    j = t % MACT
                    if j == 0:
                        hx_mac = hxp.tile([P, 1024], BF16, tag="hx")
                        nc.sync.dma_start(out=hx_mac[:], in_=hx_d[t // MACT])
                    ph = (j % 2) * 64
                    cc = (j // 2) * 512
                    ph1 = phist.tile([P, 512], F32, tag="pb1")
                    nc.tensor.matmul(out=ph1[:],
                                     lhsT=wt1b[ph:ph + 64, :],
                                     rhs=hx_mac[ph:ph + 64, cc:cc + 512],
                                     start=True, stop=True)
                    h1 = sb1.tile([P, 512], BF16, tag="h1")
                    if has_bt1:
                        nc.scalar.activation(h1[:], ph1[:], relu,
                                             bias=bt1[:, 0:1])
                    else:
                        for hh2 in range(2):
                            hs = slice(hh2 * 256, hh2 * 256 + 256)
                            nc.any.tensor_scalar_max(out=h1[:, hs],
                                                     in0=ph1[:, hs],
                                                     scalar1=0.0)
                    prm = phist.tile([P, 512], F32, tag="pb2")
                    for kk in range(4):
                        nc.tensor.matmul(
                            out=prm[:, kk * P:(kk + 1) * P],
                            lhsT=h1[:, kk * P:(kk + 1) * P], rhs=wt2b[:],
                            start=True, stop=True)
                    hrm = hrmp.tile([P, 512], BF16, tag="hrm")
                    seg4 = segp.tile([P, 512], BF16, tag="seg")
                    nc.any.tensor_tensor(
                        out=seg4[:].rearrange("p (a b) -> p a b", b=P),
                        in0=slot_sb[:, 4 * t:4 * t + 4].to_broadcast([P, 4, P]),
                        in1=iota4[:].rearrange("p (a b) -> p a b", b=P),
                        op=A.is_equal)
                    if has_bt2:
                        nc.any.tensor_tensor(out=hrm[:], in0=prm[:],
                                             in1=b2bc[:], op=A.add)
                        nc.any.tensor_scalar_max(out=hrm[:], in0=hrm[:],
                                                 scalar1=0.0)
                    else:
                        for hh2 in range(2):
                            hs = slice(hh2 * 256, hh2 * 256 + 256)
                            nc.any.tensor_scalar_max(out=hrm[:, hs],
                                                     in0=prm[:, hs],
                                                     scalar1=0.0)
                    for kk in range(4):
                        chunk = 4 * t + kk
                        w, ci = divmod(chunk, CPW)
                        qs = slice(kk * P, (kk + 1) * P)
                        if ci == 0 and w % 4 == 0:
                            pseg_t = pseg.tile([P, 512], F32, tag="ps")
                        nc.tensor.matmul(
                            out=pseg_t[:, (w % 4) * P:(w % 4 + 1) * P],
                            lhsT=hrm[:, qs], rhs=seg4[:, qs],
                            start=(ci == 0), stop=(ci == CPW - 1))
                        if ci == CPW - 1 and w % 4 == 3:
                            g = w // 4
                            if g % 2 == 0:
                                pre_mac = prep.tile([P, 1024], F32, tag="pre")
                                nc.sync.dma_start(out=pre_mac[:],
                                                  in_=pre_d[g // 2])
                            card_group(g, pseg_t, pre_mac)

            # ---------------- target MLP
            for t in range(TT):
                j = t % MACT
                if j == 0:
                    tx_mac = hxp.tile([P, 1024], F32, tag="hx")
                    nc.sync.dma_start(out=tx_mac[:], in_=tx_d[t // MACT])
                ph = (j % 2) * 64
                cc = (j // 2) * 512
                ph1 = pbig.tile([P, 512], F32, tag="pb")
                nc.tensor.matmul(out=ph1[:], lhsT=wt1d[ph:ph + 64, :],
                                 rhs=tx_mac[ph:ph + 64, cc:cc + 512],
                                 start=True, stop=True)
                h1 = sb1.tile([P, 512], F32, tag="h1")
                nc.scalar.activation(h1[:], ph1[:], relu, bias=bt1[:, 0:1])
                ph2 = pbig.tile([P, 512], F32, tag="pb")
                nc.tensor.matmul(out=ph2[:], lhsT=wt2[:], rhs=h1[:],
                                 start=True, stop=True)
                nc.scalar.activation(tgt_h[:, t * 512:(t + 1) * 512], ph2[:],
                                     relu, bias=bt2[:, 0:1])

            # ---------------- gather + head
            with (tc.tile_pool(name="pgat", bufs=2, space="PSUM") as pgat,
                  tc.tile_pool(name="plog", bufs=2, space="PSUM") as plog,
                  tc.tile_pool(name="headp", bufs=3) as headp,
                  tc.tile_pool(name="selp", bufs=4) as selp):
                for g in range(TT):
                    tb = headp.tile([P, 512], F32, tag="tb")
                    nc.sync.dma_start(
                        out=tb[:],
                        in_=trel_d[0:1, g * 512:(g + 1) * 512]
                        .to_broadcast([P, 512]))
                    tch = headp.tile([P, 512], F32, tag="tch")
                    for kk in range(4):
                        c = 4 * g + kk
                        pg = pgat.tile([P, P], F32, tag="pg")
                        for jj in range(TGT_J):
                            sel = selp.tile([P, P], BF16, tag="sel")
                            nc.any.tensor_scalar(
                                out=sel[:],
                                in0=tb[:, kk * P:(kk + 1) * P],
                                scalar1=iota_c[:, 0:1],
                                scalar2=float(P * jj),
                                op0=A.subtract,
                                op1=A.is_equal)
                            nc.tensor.matmul(
                                out=pg[:],
                                lhsT=card_h[:, (2 * c + jj) * P:(2 * c + jj + 1) * P],
                                rhs=sel[:],
                                start=(jj == 0), stop=(jj == TGT_J - 1))
                        nc.any.tensor_copy(
                            out=tch[:, kk * P:(kk + 1) * P], in_=pg[:])
                    phh = pbig.tile([P, 512], F32, tag="pb")
                    nc.tensor.matmul(out=phh[:], lhsT=wh1a[:],
                                     rhs=tgt_h[:, g * 512:(g + 1) * 512],
                                     start=True, stop=False)
                    nc.tensor.matmul(out=phh[:], lhsT=wh1b[:], rhs=tch[:],
                                     start=False, stop=True)
                    hh = headp.tile([P, 512], F32, tag="hh")
                    if has_bh1:
                        nc.scalar.activation(hh[:], phh[:], relu,
                                             bias=bh1[:, 0:1])
                    else:
                        nc.any.tensor_scalar_max(out=hh[:], in0=phh[:],
                                                 scalar1=0.0)
                    pl = plog.tile([1, 512], F32, tag="pl")
                    nc.tensor.matmul(out=pl[:], lhsT=wh2[:], rhs=hh[:],
                                     start=True, stop=True)
                    ol = headp.tile([1, 512], F32, tag="ol")
                    nc.scalar.activation(ol[:], pl[:], fcopy,
                                         bias=float(weights["b_h2"][0, 0]))
                    nc.sync.dma_start(
                        out=out_d[0:1, g * 512:(g + 1) * 512], in_=ol[:])
    nc.finalize()
    return nc


# ------------------------------------------------------------------ driver

def _weights_dict(W_t1, b_t1, W_t2, b_t2, W_c1, b_c1, W_c2, b_c2,
                  W_h1, b_h1, W_h2, b_h2, CF):
    import ml_dtypes
    col = lambda v: np.ascontiguousarray(v.reshape(-1, 1).astype(np.float32))
    iota = np.arange(P, dtype=np.float32)
    w = dict(
        wt1d=np.ascontiguousarray(
            np.vstack([W_t1, W_t1]).astype(np.float32)),
        wt1b=np.ascontiguousarray(
            np.vstack([W_t1, W_t1]).astype(ml_dtypes.bfloat16)),
        wt2b=np.ascontiguousarray(
            np.asarray(W_t2).astype(ml_dtypes.bfloat16)),
        b_t1=col(b_t1),
        W_t2=np.ascontiguousarray(W_t2, np.float32),
        b_t2=col(b_t2),
        W_c1b=np.ascontiguousarray(W_c1[CF:], np.float32),
        W_c2=np.ascontiguousarray(W_c2, np.float32),
        W_h1a=np.ascontiguousarray(W_h1[:D], np.float32),
        W_h1b=np.ascontiguousarray(W_h1[D:], np.float32),
        b_h1=col(b_h1),
        W_h2=np.ascontiguousarray(W_h2, np.float32),
        b_h2=np.array([[float(np.asarray(b_h2).reshape(-1)[0])]], np.float32),
        iota4=np.ascontiguousarray(np.tile(iota[None, :], (P, 4))),
        iota_c=col(iota),
    )
    has_bt2 = bool(np.any(np.asarray(b_t2) != 0))
    has_bc2 = bool(np.any(np.asarray(b_c2) != 0))
    if has_bt2:
        w["b2bc"] = np.ascontiguousarray(
            np.tile(np.asarray(b_t2, np.float32).reshape(1, -1), (P, 4)))
    if has_bc2:
        w["bc2bc"] = np.ascontiguousarray(
            np.tile(np.asarray(b_c2, np.float32).reshape(1, -1), (P, 4)))
    return w, has_bt2, has_bc2


def _prepare(inputs):
    gi = lambda n: np.asarray(inputs[n])
    target_x = gi("target_x").astype(np.float32)
    hist_x = gi("hist_x").astype(np.float32)
    hist_idx = gi("hist_card_local_idx").astype(np.int64)
    target_idx = gi("target_card_local_idx").astype(np.int64)
    card_dense = gi("card_dense_feats").astype(np.float32)
    CF = card_dense.shape[1]
    weights, has_bt2, has_bc2 = _weights_dict(
        gi("W_t1"), gi("b_t1"), gi("W_t2"), gi("b_t2"),
        gi("W_c1"), gi("b_c1"), gi("W_c2"), gi("b_c2"),
        gi("W_h1"), gi("b_h1"), gi("W_h2"), gi("b_h2"), CF)
    in_maps, scatter, meta = _plan(
        target_x, hist_x, hist_idx, target_idx, card_dense,
        gi("W_c1").astype(np.float32), gi("b_c1").astype(np.float32))
    meta["has_bt2"] = has_bt2
    meta["has_bc2"] = has_bc2
    meta["has_bt1"] = bool(np.any(np.asarray(inputs["b_t1"]) != 0))
    meta["has_bh1"] = bool(np.any(np.asarray(inputs["b_h1"]) != 0))
    for m in in_maps:
        m.update(weights)
    return in_maps, scatter, meta, weights, target_x.shape[0]


def kernel(**inputs):
    in_maps, scatter, meta, weights, B = _prepare(inputs)
    nc = _build(meta, weights)

    if os.environ.get("BASS_KERNEL_SIM"):
        from concourse import bass_interp
        outs = []
        for k in range(NCORES):
            sim = bass_interp.CoreSim(nc)
            for name, arr in in_maps[k].items():
                sim.tensor(name)[:] = arr
            sim.simulate()
            outs.append(np.array(sim.tensor("logits")))
    else:
        from concourse.bass_utils import run_bass_kernel_spmd
        res = run_bass_kernel_spmd(nc, in_maps, list(range(NCORES)))
        outs = [res.results[k]["logits"] for k in range(NCORES)]

    out = np.zeros(B, np.float32)
    for k in range(NCORES):
        ids, pos = scatter[k]
        out[ids] = outs[k].reshape(-1)[pos]
    return out
